# revision 6
# baseline (speedup 1.0000x reference)
"""Trainium2 Bass kernel for a pre-LN transformer block (B=2, S=2048, D=1024,
H=16 heads, d_ff=4096), data-parallel over 8 NeuronCores.

Sharding: each core owns 512 query tokens (2 batches x 4 blocks).  Every core
LayerNorms the full 2048-token sequence of its batch and computes K/V for all
of it (redundant K/V, zero communication); Q / attention / O-proj / FFN are
computed only for the core's 512 tokens.  The per-core token axis is rolled on
the host so the core's query block always sits at positions 0:512 — attention
is permutation-invariant over keys, so K/V order does not matter as long as
the mask bias is rolled the same way.

On-chip dataflow is entirely feature-major ([d_model, tokens]); LayerNorm
statistics are computed with ones-vector matmuls on the PE, per-token scalars
are broadcast across partitions with K=1 outer-product matmuls, and softmax
denominators come from an extra all-ones column appended to V.  Matmul inputs
are bf16 (weights pre-cast on host), accumulation fp32 in PSUM; residuals,
LayerNorm and softmax math are fp32.
"""

import numpy as np
import ml_dtypes

import concourse.bass as bass
import concourse.mybir as mybir
import concourse.tile as tile
from concourse.bass import ts
from concourse.vector_clock import ScopedClock
from concourse.bass_utils import run_bass_kernel_spmd
from concourse.masks import make_identity

AF = mybir.ActivationFunctionType
ALU = mybir.AluOpType
FP32 = mybir.dt.float32
BF16 = mybir.dt.bfloat16

P = 128
D = 1024
H = 16
DH = 64
DFF = 4096
TQ = 512          # query tokens per core
TKV = 2048        # keys/values per core (full batch sequence)
NDC = D // P      # 8 d_model chunks
NFC = DFF // P    # 32 d_ff chunks
NKC = TKV // P    # 16 key chunks
NCORES = 8
EPS = 1e-6
SOFTMAX_SHIFT = 4.0   # subtracted from scores pre-exp (cancels in the ratio)

# This container's walrus build accepts at most ONE sync-wait per instruction.
MAXW = 1


class _TileCtx(tile.TileContext):
    """TileContext whose exit drain splits its sem waits across a chain of
    drains (walrus here rejects instructions with >MAXW sync waits)."""

    def _drain_and_barrier(self, tick_clock, wait_clock):
        probe = self.nc.sync.drain()
        wait_clock.add_sem_waits(probe.ins, ScopedClock({None: tick_clock.global_clock}))
        si = probe.ins.sync_info
        waits = list(si.on_wait) if si is not None else []
        if len(waits) > MAXW:
            probe.ins.sync_info = mybir.SyncInfo(
                on_wait=waits[:MAXW], on_update=list(si.on_update)
            )
            for i in range(MAXW, len(waits), MAXW):
                d = self.nc.sync.drain()
                d.ins.sync_info = mybir.SyncInfo(on_wait=waits[i : i + MAXW], on_update=[])
        self.nc.all_engine_barrier()
        assert self.sems is not None
        popped = self.nc._tile_sem_poison_stack.pop()
        assert popped is self._sem_poison
        self.nc.clear_and_free_semaphores(list(self.sems.allocated().values()))
        self.nc.all_engine_barrier()


def _split_multi_waits(nc, maxw=MAXW):
    """Hoist extra sem waits of any instruction onto preceding same-engine
    nops (same queue => in-order => semantically identical)."""
    n_split = 0
    for f in nc.m.functions:
        for blk in f.blocks:
            insts = blk.instructions
            i = 0
            while i < len(insts):
                inst = insts[i]
                si = getattr(inst, "sync_info", None)
                if si is not None and len(si.on_wait) > maxw:
                    waits = list(si.on_wait)
                    extra = waits[maxw:]
                    carriers = []
                    for j in range(0, len(extra), maxw):
                        nop = mybir.InstNoOp(name=f"{inst.name}-wsplit{j}", ins=[], outs=[])
                        nop.engine = inst.engine
                        nop.sync_info = mybir.SyncInfo(on_wait=extra[j : j + maxw], on_update=[])
                        carriers.append(nop)
                    inst.sync_info = mybir.SyncInfo(
                        on_wait=waits[:maxw], on_update=list(si.on_update)
                    )
                    for k, c in enumerate(carriers):
                        insts.insert(i + k, c)
                    i += len(carriers)
                    n_split += 1
                i += 1
    return n_split


class _Pools:
    def __init__(self, tc):
        self.tc = tc
        self._cms = {}
        self._order = []

    def open(self, name, bufs=1, space="SBUF", side=None):
        kw = dict(name=name, bufs=bufs, space=space)
        if side is not None:
            kw["side"] = side
        cm = self.tc.tile_pool(**kw)
        pool = cm.__enter__()
        self._cms[name] = (cm, pool)
        self._order.append(name)
        return pool

    def close(self, name):
        cm, _ = self._cms.pop(name)
        self._order.remove(name)
        cm.__exit__(None, None, None)

    def close_all(self):
        for name in reversed(list(self._order)):
            self.close(name)


def _emit(nc, t):
    with _TileCtx(nc) as tc:
        pools = _Pools(tc)

        ps_mm = pools.open("ps_mm", bufs=2, space="PSUM")
        ps_st = pools.open("ps_st", bufs=2, space="PSUM")
        ps_pa = pools.open("ps_pa", bufs=2, space="PSUM")
        ps_bc = pools.open("ps_bc", bufs=2, space="PSUM")

        # ---------------- constants / params ----------------
        const = pools.open("const", bufs=1)

        ones_col = const.tile([P, 1], BF16, name="ones_col")
        nc.vector.memset(ones_col[:], 1.0)
        ones_row = const.tile([1, P], FP32, name="ones_row")
        nc.vector.memset(ones_row[:], 1.0)
        ident = const.tile([P, P], FP32, name="ident")
        make_identity(nc, ident[:])

        def load_pf(ap_dram, n_chunks, name):
            tl = const.tile([P, n_chunks], FP32, name=name)
            nc.sync.dma_start(tl[:], ap_dram.rearrange("(c p) -> p c", p=P))
            return tl

        g1 = load_pf(t["ln1_g"], NDC, "g1")
        b1l = load_pf(t["ln1_b"], NDC, "b1l")
        g2 = load_pf(t["ln2_g"], NDC, "g2")
        b2l = load_pf(t["ln2_b"], NDC, "b2l")
        bq_s = load_pf(t["bq"], NDC, "bq_s")
        nc.vector.tensor_scalar_mul(bq_s[:], bq_s[:], 0.125)
        bk_c = load_pf(t["bk"], NDC, "bk_c")
        bo_c = load_pf(t["bo"], NDC, "bo_c")
        b2_c = load_pf(t["b2"], NDC, "b2_c")
        b1_c = load_pf(t["b1"], NFC, "b1_c")
        mbt = load_pf(t["mb"], NKC, "mbt")

        # bv broadcast to all partitions: [1, D] -> [P, D]
        bv_row = const.tile([1, D], FP32, name="bv_row")
        nc.sync.dma_start(bv_row[:], t["bv"].rearrange("(a d) -> a d", a=1))
        bvb = const.tile([P, D], FP32, name="bvb")
        for half in range(2):
            pb = ps_bc.tile([P, 512], FP32, tag="bc")
            nc.tensor.matmul(pb[:], ones_row[:], bv_row[:, ts(half, 512)], start=True, stop=True)
            nc.scalar.copy(bvb[:, ts(half, 512)], pb[:])

        def bcast(row_ap, out_ap, width, parts=P):
            """out[p, :w] = row[0, :w] for p in range(parts), via K=1 matmul."""
            nwhole = width // 512
            for i in range(nwhole):
                pb = ps_bc.tile([P, 512], FP32, tag="bc")
                nc.tensor.matmul(pb[0:parts, :], ones_row[:, 0:parts], row_ap[:, ts(i, 512)],
                                 start=True, stop=True)
                nc.scalar.copy(out_ap[:, ts(i, 512)], pb[0:parts, :])

        # ---------------- phase A: LN1 over TKV tokens ----------------
        pst = pools.open("stsc", bufs=1)
        phq = pools.open("hq", bufs=1)
        ph = pools.open("h", bufs=1)
        pxT = pools.open("xT", bufs=1)
        plnA = pools.open("lnA", bufs=2)

        xT_t = pxT.tile([P, NDC, TKV], FP32, name="xTt")
        xT_r = t["xT"].rearrange("(c p) k -> p c k", p=P)
        for c in range(NDC):
            nc.sync.dma_start(xT_t[:, c, :], xT_r[:, c, :])

        def ln_tile(src3, tok_sl, dst_bf, dst_f32, g, b, spool, tagp):
            """LayerNorm 512 tokens of src3 ([P, NDC, ntok] fp32) at slice
            tok_sl; write bf16 result to dst_bf[:, c, :] slices and optionally
            fp32 to dst_f32.  ddof=1, denominator (std + eps)."""
            ps = ps_st.tile([33, 512], FP32, tag="st")
            for c in range(NDC):
                cast = spool.tile([P, 1024], BF16, tag=f"cast{tagp}")
                nc.scalar.copy(cast[:, 0:512], src3[:, c, tok_sl])
                nc.tensor.matmul(ps[0:1, :], ones_col[:], cast[:, 0:512],
                                 start=(c == 0), stop=(c == NDC - 1))
                nc.scalar.activation(cast[:, 512:1024], src3[:, c, tok_sl], AF.Square)
                nc.tensor.matmul(ps[32:33, :], ones_col[:], cast[:, 512:1024],
                                 start=(c == 0), stop=(c == NDC - 1))
            mean = pst.tile([1, 512], FP32, tag="mean")
            nc.vector.tensor_scalar_mul(mean[:], ps[0:1, :], 1.0 / D)
            var = pst.tile([1, 512], FP32, tag="var")
            nc.vector.tensor_scalar_mul(var[:], ps[32:33, :], 1.0 / (D - 1))
            msq = pst.tile([1, 512], FP32, tag="msq")
            nc.vector.tensor_mul(msq[:], mean[:], mean[:])
            nc.vector.tensor_scalar(msq[:], msq[:], float(D) / (D - 1), None, ALU.mult)
            nc.vector.tensor_tensor(var[:], var[:], msq[:], ALU.subtract)
            std = pst.tile([1, 512], FP32, tag="std")
            nc.scalar.activation(std[:], var[:], AF.Sqrt)
            nc.vector.tensor_scalar_add(std[:], std[:], EPS)
            rrow = pst.tile([1, 512], FP32, tag="rrow")
            nc.vector.reciprocal(rrow[:], std[:])
            nmrow = pst.tile([1, 512], FP32, tag="nmrow")
            nc.vector.tensor_mul(nmrow[:], mean[:], rrow[:])
            # broadcast across partitions via K=1 matmuls; apply from PSUM
            pb_r = ps_bc.tile([P, 512], FP32, tag="bc")
            nc.tensor.matmul(pb_r[:], ones_row[:], rrow[:], start=True, stop=True)
            pb_n = ps_bc.tile([P, 512], FP32, tag="bc")
            nc.tensor.matmul(pb_n[:], ones_row[:], nmrow[:], start=True, stop=True)
            for c in range(NDC):
                tmp = spool.tile([P, 512], FP32, tag=f"lntmp{tagp}")
                nc.vector.tensor_mul(tmp[:], src3[:, c, tok_sl], pb_r[:])
                nc.vector.tensor_tensor(tmp[:], tmp[:], pb_n[:], ALU.subtract)
                nc.vector.tensor_scalar(dst_bf[:, c, tok_sl], tmp[:], g[:, c : c + 1],
                                        b[:, c : c + 1], ALU.mult, ALU.add)
                if dst_f32 is not None:
                    nc.vector.tensor_scalar(dst_f32[:, c, :], tmp[:], g[:, c : c + 1],
                                            b[:, c : c + 1], ALU.mult, ALU.add)

        hT = ph.tile([P, NDC, TKV], BF16, name="hT")
        hq = phq.tile([P, NDC, TQ], FP32, name="hq")
        for tt in range(TKV // 512):
            ln_tile(xT_t, ts(tt, 512), hT, hq if tt == 0 else None, g1, b1l, plnA, "A")
        pools.close("lnA")
        pools.close("xT")

        # ---------------- phase B: Q/K/V projections ----------------
        pw = pools.open("wproj", bufs=2)
        pqkv = pools.open("qkv", bufs=1, side="right")

        wq_t = pw.tile([P, NDC, D], BF16, tag="wm")
        nc.sync.dma_start(wq_t[:], t["wq"].rearrange("(c p) n -> p c n", p=P))
        qT = pqkv.tile([P, NDC, TQ], BF16, name="qT")
        for n in range(NDC):
            ps = ps_mm.tile([P, 512], FP32, tag="mm")
            for c in range(NDC):
                nc.tensor.matmul(ps[:], wq_t[:, c, ts(n, P)], hT[:, c, 0:TQ],
                                 start=(c == 0), stop=(c == NDC - 1))
            nc.scalar.activation(qT[:, n, :], ps[:], AF.Identity,
                                 bias=bq_s[:, n : n + 1], scale=0.125)

        wk_t = pw.tile([P, NDC, D], BF16, tag="wm")
        nc.sync.dma_start(wk_t[:], t["wk"].rearrange("(c p) n -> p c n", p=P))
        kT = pqkv.tile([P, NDC, TKV], BF16, name="kT")
        for n in range(NDC):
            for tt in range(TKV // 512):
                ps = ps_mm.tile([P, 512], FP32, tag="mm")
                for c in range(NDC):
                    nc.tensor.matmul(ps[:], wk_t[:, c, ts(n, P)], hT[:, c, ts(tt, 512)],
                                     start=(c == 0), stop=(c == NDC - 1))
                nc.scalar.activation(kT[:, n, ts(tt, 512)], ps[:], AF.Identity,
                                     bias=bk_c[:, n : n + 1], scale=1.0)

        wv_t = pw.tile([P, NDC, D], BF16, tag="wm")
        nc.sync.dma_start(wv_t[:], t["wv"].rearrange("(c p) n -> p c n", p=P))
        vaug = pqkv.tile([P, NKC, H, DH + 1], BF16, name="vaug")
        nc.vector.memset(vaug[:, :, :, DH], 1.0)
        for kc in range(NKC):
            for half in range(2):
                ps = ps_mm.tile([P, 512], FP32, tag="mm")
                for c in range(NDC):
                    nc.tensor.matmul(ps[:], hT[:, c, ts(kc, P)], wv_t[:, c, ts(half, 512)],
                                     start=(c == 0), stop=(c == NDC - 1))
                nc.vector.tensor_tensor(
                    vaug[:, kc, 8 * half : 8 * half + 8, 0:DH],
                    ps[:].rearrange("p (h d) -> p h d", d=DH),
                    bvb[:, ts(half, 512)].rearrange("p (h d) -> p h d", d=DH),
                    ALU.add,
                )
        pools.close("wproj")
        pools.close("h")

        # wo prefetch overlaps attention (released wproj zone gates the DMA)
        pwo = pools.open("wo", bufs=1)
        wo_t = pwo.tile([P, NDC, D], BF16, name="wo_t")
        nc.sync.dma_start(wo_t[:], t["wo"].rearrange("(c p) n -> p c n", p=P))

        # ---------------- phase C: attention (per head, sw-pipelined) ----------------
        ppt = pools.open("pt", bufs=2, side="right")
        pattn = pools.open("attn", bufs=1)
        attnT = pattn.tile([P, NDC, TQ], BF16, name="attnT")

        pt_tiles = [None] * H

        def emit_scores(h):
            pof = DH * (h % 2)
            ch = h // 2
            pt = ppt.tile([P, NKC, TQ], BF16, tag="pt")
            pt_tiles[h] = pt
            for kc in range(NKC):
                ps = ps_mm.tile([P, 512], FP32, tag="mm")
                nc.tensor.matmul(ps[:], kT[pof : pof + DH, ch, ts(kc, P)],
                                 qT[pof : pof + DH, ch, :], start=True, stop=True)
                nc.scalar.activation(pt[:, kc, :], ps[:], AF.Exp,
                                     bias=mbt[:, kc : kc + 1], scale=1.0)

        def emit_attnv(h):
            pof = DH * (h % 2)
            ch = h // 2
            pt = pt_tiles[h]
            pa = ps_pa.tile([DH + 1, TQ], FP32, tag="pa")
            for kc in range(NKC):
                nc.tensor.matmul(pa[:], vaug[:, kc, h, :], pt[:, kc, :],
                                 start=(kc == 0), stop=(kc == NKC - 1))
            rd = pattn.tile([1, TQ], FP32, tag="rd")
            nc.vector.reciprocal(rd[:], pa[DH : DH + 1, :])
            pb = ps_bc.tile([P, 512], FP32, tag="bc")
            nc.tensor.matmul(pb[0:DH, :], ones_row[:, 0:DH], rd[:], start=True, stop=True)
            rdb = pattn.tile([DH, TQ], FP32, tag="rdb")
            nc.scalar.copy(rdb[:], pb[0:DH, :])
            nc.vector.tensor_mul(attnT[pof : pof + DH, ch, :], pa[0:DH, :], rdb[:])

        # software pipeline: scores(h+1) overlaps exp(h) / attnV(h)
        emit_scores(0)
        for h in range(H):
            if h + 1 < H:
                emit_scores(h + 1)
            emit_attnv(h)
            pt_tiles[h] = None
        pools.close("pt")
        pools.close("qkv")

        # ---------------- phase D: output projection + residual ----------------
        pres = pools.open("res1", bufs=1, side="right")
        res1T = pres.tile([P, NDC, TQ], FP32, name="res1T")
        for n in range(NDC):
            ps = ps_mm.tile([P, 512], FP32, tag="mm")
            for c in range(NDC):
                nc.tensor.matmul(ps[:], wo_t[:, c, ts(n, P)], attnT[:, c, :],
                                 start=(c == 0), stop=(c == NDC - 1))
            nc.vector.tensor_scalar(res1T[:, n, :], ps[:], bo_c[:, n : n + 1], None, ALU.add)
            nc.vector.tensor_add(res1T[:, n, :], res1T[:, n, :], hq[:, n, :])
        pools.close("attn")
        pools.close("wo")
        pools.close("hq")

        # ---------------- phase E: LN2 (TQ tokens) ----------------
        plnE = pools.open("lnE", bufs=2, side="right")
        h2T = plnE.tile([P, NDC, TQ], BF16, name="h2T")
        ln_tile(res1T, slice(0, TQ), h2T, None, g2, b2l, plnE, "E")

        # ---------------- phase F: FFN (interleaved halves) ----------------
        pg1 = pools.open("g1", bufs=1)
        pwf1 = pools.open("wf1", bufs=1)
        pwf2 = pools.open("wf2", bufs=1)
        pout = pools.open("out", bufs=1)

        g1T = pg1.tile([P, NFC, TQ], BF16, name="g1T")
        outT = pout.tile([P, NDC, TQ], FP32, name="outT")
        w1_r = t["w1"].rearrange("(c p) f -> p c f", p=P)
        w2_r = t["w2"].rearrange("(f p) d -> p f d", p=P)

        for hf in range(2):
            w1h = pwf1.tile([P, NDC, DFF // 2], BF16, tag="w1h")
            nc.sync.dma_start(w1h[:], w1_r[:, :, ts(hf, DFF // 2)])
            for fl in range(NFC // 2):
                fc = hf * (NFC // 2) + fl
                ps = ps_mm.tile([P, 512], FP32, tag="mm")
                for c in range(NDC):
                    nc.tensor.matmul(ps[:], w1h[:, c, ts(fl, P)], h2T[:, c, :],
                                     start=(c == 0), stop=(c == NDC - 1))
                nc.scalar.activation(g1T[:, fc, :], ps[:], AF.Relu,
                                     bias=b1_c[:, fc : fc + 1], scale=1.0)
            # second FFN matmul for this half of d_ff
            w2h = pwf2.tile([P, NFC // 2, D], BF16, tag="w2h")
            nc.sync.dma_start(w2h[:], w2_r[:, ts(hf, NFC // 2), :])
            for n in range(NDC):
                ps = ps_mm.tile([P, 512], FP32, tag="mm")
                for fl in range(NFC // 2):
                    fc = hf * (NFC // 2) + fl
                    nc.tensor.matmul(ps[:], w2h[:, fl, ts(n, P)], g1T[:, fc, :],
                                     start=(fl == 0), stop=(fl == NFC // 2 - 1))
                if hf == 0:
                    nc.vector.tensor_scalar(outT[:, n, :], ps[:], b2_c[:, n : n + 1],
                                            None, ALU.add)
                else:
                    nc.vector.tensor_add(outT[:, n, :], outT[:, n, :], ps[:])
        pools.close("lnE")

        # final residual: out = res1 + ffn
        for n in range(NDC):
            nc.vector.tensor_add(outT[:, n, :], outT[:, n, :], res1T[:, n, :])
        pools.close("res1")

        # ---------------- transpose to token-major and store ----------------
        out_sb = pout.tile([P, TQ // P, D], FP32, name="out_sb")
        for n in range(NDC):
            for qc in range(TQ // P):
                pt_ = ps_bc.tile([P, P], FP32, tag="bc")
                nc.tensor.transpose(pt_[:], outT[:, n, ts(qc, P)], ident[:])
                nc.scalar.copy(out_sb[:, qc, ts(n, P)], pt_[:])
        nc.sync.dma_start(t["out"].rearrange("(qc p) d -> p qc d", p=P), out_sb[:])

        pools.close_all()


def _build_nc(split=True):
    nc = bass.Bass("TRN2", target_bir_lowering=False, debug=False)

    t = {}

    def inp(name, shape, dtype=FP32):
        t[name] = nc.dram_tensor(name, shape, dtype, kind="ExternalInput").ap()

    inp("xT", [D, TKV])
    inp("mb", [TKV])
    for nm in ("ln1_g", "ln1_b", "ln2_g", "ln2_b", "bq", "bk", "bv", "bo", "b2"):
        inp(nm, [D])
    inp("b1", [DFF])
    for nm in ("wq", "wk", "wv", "wo"):
        inp(nm, [D, D], BF16)
    inp("w1", [D, DFF], BF16)
    inp("w2", [DFF, D], BF16)
    t["out"] = nc.dram_tensor("out", [TQ, D], FP32, kind="ExternalOutput").ap()

    _emit(nc, t)
    if split:
        _split_multi_waits(nc)
    return nc


_NC_CACHE = None


def _get_nc():
    global _NC_CACHE
    if _NC_CACHE is None:
        _NC_CACHE = _build_nc()
    return _NC_CACHE


def make_in_maps(x, mask, ln1_g, ln1_b, ln2_g, ln2_b,
                 wq, bq, wk, bk, wv, bv, wo, bo, w1, b1, w2, b2):
    """Build the 8 per-core input dicts from the full-size inputs."""
    bf = ml_dtypes.bfloat16
    shared = {
        "ln1_g": np.ascontiguousarray(ln1_g, np.float32),
        "ln1_b": np.ascontiguousarray(ln1_b, np.float32),
        "ln2_g": np.ascontiguousarray(ln2_g, np.float32),
        "ln2_b": np.ascontiguousarray(ln2_b, np.float32),
        "bq": np.ascontiguousarray(bq, np.float32),
        "bk": np.ascontiguousarray(bk, np.float32),
        "bv": np.ascontiguousarray(bv, np.float32),
        "bo": np.ascontiguousarray(bo, np.float32),
        "b1": np.ascontiguousarray(b1, np.float32),
        "b2": np.ascontiguousarray(b2, np.float32),
        "wq": np.ascontiguousarray(np.asarray(wq, np.float32).astype(bf)),
        "wk": np.ascontiguousarray(np.asarray(wk, np.float32).astype(bf)),
        "wv": np.ascontiguousarray(np.asarray(wv, np.float32).astype(bf)),
        "wo": np.ascontiguousarray(np.asarray(wo, np.float32).astype(bf)),
        "w1": np.ascontiguousarray(np.asarray(w1, np.float32).astype(bf)),
        "w2": np.ascontiguousarray(np.asarray(w2, np.float32).astype(bf)),
    }
    x = np.asarray(x, np.float32)
    mask = np.asarray(mask)
    in_maps = []
    for core in range(NCORES):
        b, qb = divmod(core, NCORES // 2)
        shift = -qb * TQ
        xb = np.roll(x[b], shift, axis=0)                      # (TKV, D)
        mb = np.roll(np.where(mask[b] == 0, -1e9, 0.0).astype(np.float32), shift)
        mb = mb - SOFTMAX_SHIFT
        in_maps.append({
            **shared,
            "xT": np.ascontiguousarray(xb.T),
            "mb": np.ascontiguousarray(mb),
        })
    return in_maps


def kernel(**inputs):
    nc = _get_nc()
    in_maps = make_in_maps(**inputs)
    res = run_bass_kernel_spmd(nc, in_maps, core_ids=list(range(NCORES)))
    Bsz, S, _ = np.asarray(inputs["x"]).shape
    out = np.empty((Bsz, S, D), np.float32)
    for core in range(NCORES):
        b, qb = divmod(core, NCORES // 2)
        out[b, qb * TQ : (qb + 1) * TQ, :] = res.results[core]["out"]
    return out


# revision 8
# speedup vs baseline: 1.0704x; 1.0704x over previous
"""Trainium2 Bass kernel for a pre-LN transformer block (B=2, S=2048, D=1024,
H=16 heads, d_ff=4096), data-parallel over 8 NeuronCores.

Sharding: each core owns 512 query tokens (2 batches x 4 blocks).  Every core
LayerNorms the full 2048-token sequence of its batch and computes K/V for all
of it (redundant K/V, zero communication); Q / attention / O-proj / FFN are
computed only for the core's 512 tokens.  The per-core token axis is rolled on
the host so the core's query block always sits at positions 0:512 — attention
is permutation-invariant over keys, so K/V order does not matter as long as
the mask bias is rolled the same way.

On-chip dataflow is entirely feature-major ([d_model, tokens]); LayerNorm
statistics are computed with ones-vector matmuls on the PE, per-token scalars
are broadcast across partitions with K=1 outer-product matmuls, and softmax
denominators come from an extra all-ones column appended to V.  Matmul inputs
are bf16 (weights pre-cast on host), accumulation fp32 in PSUM; residuals,
LayerNorm and softmax math are fp32.
"""

import numpy as np
import ml_dtypes

import concourse.bass as bass
import concourse.mybir as mybir
import concourse.tile as tile
from concourse.bass import ts
from concourse.vector_clock import ScopedClock
from concourse.bass_utils import run_bass_kernel_spmd
from concourse.masks import make_identity

AF = mybir.ActivationFunctionType
ALU = mybir.AluOpType
FP32 = mybir.dt.float32
BF16 = mybir.dt.bfloat16

P = 128
D = 1024
H = 16
DH = 64
DFF = 4096
TQ = 512          # query tokens per core
TKV = 2048        # keys/values per core (full batch sequence)
NDC = D // P      # 8 d_model chunks
NFC = DFF // P    # 32 d_ff chunks
NKC = TKV // P    # 16 key chunks
NCORES = 8
EPS = 1e-6

# This container's walrus build accepts at most ONE sync-wait per instruction.
MAXW = 1


class _TileCtx(tile.TileContext):
    """TileContext whose exit drain splits its sem waits across a chain of
    drains (walrus here rejects instructions with >MAXW sync waits)."""

    def _drain_and_barrier(self, tick_clock, wait_clock):
        probe = self.nc.sync.drain()
        wait_clock.add_sem_waits(probe.ins, ScopedClock({None: tick_clock.global_clock}))
        si = probe.ins.sync_info
        waits = list(si.on_wait) if si is not None else []
        if len(waits) > MAXW:
            probe.ins.sync_info = mybir.SyncInfo(
                on_wait=waits[:MAXW], on_update=list(si.on_update)
            )
            for i in range(MAXW, len(waits), MAXW):
                d = self.nc.sync.drain()
                d.ins.sync_info = mybir.SyncInfo(on_wait=waits[i : i + MAXW], on_update=[])
        self.nc.all_engine_barrier()
        assert self.sems is not None
        popped = self.nc._tile_sem_poison_stack.pop()
        assert popped is self._sem_poison
        self.nc.clear_and_free_semaphores(list(self.sems.allocated().values()))
        self.nc.all_engine_barrier()


def _split_multi_waits(nc, maxw=MAXW):
    """Hoist extra sem waits of any instruction onto preceding same-engine
    nops (same queue => in-order => semantically identical)."""
    n_split = 0
    for f in nc.m.functions:
        for blk in f.blocks:
            insts = blk.instructions
            i = 0
            while i < len(insts):
                inst = insts[i]
                si = getattr(inst, "sync_info", None)
                if si is not None and len(si.on_wait) > maxw:
                    waits = list(si.on_wait)
                    extra = waits[maxw:]
                    carriers = []
                    for j in range(0, len(extra), maxw):
                        nop = mybir.InstNoOp(name=f"{inst.name}-wsplit{j}", ins=[], outs=[])
                        nop.engine = inst.engine
                        nop.sync_info = mybir.SyncInfo(on_wait=extra[j : j + maxw], on_update=[])
                        carriers.append(nop)
                    inst.sync_info = mybir.SyncInfo(
                        on_wait=waits[:maxw], on_update=list(si.on_update)
                    )
                    for k, c in enumerate(carriers):
                        insts.insert(i + k, c)
                    i += len(carriers)
                    n_split += 1
                i += 1
    return n_split


class _Pools:
    def __init__(self, tc):
        self.tc = tc
        self._cms = {}
        self._order = []

    def open(self, name, bufs=1, space="SBUF", side=None):
        kw = dict(name=name, bufs=bufs, space=space)
        if side is not None:
            kw["side"] = side
        cm = self.tc.tile_pool(**kw)
        pool = cm.__enter__()
        self._cms[name] = (cm, pool)
        self._order.append(name)
        return pool

    def close(self, name):
        cm, _ = self._cms.pop(name)
        self._order.remove(name)
        cm.__exit__(None, None, None)

    def close_all(self):
        for name in reversed(list(self._order)):
            self.close(name)


def _emit(nc, t):
    with _TileCtx(nc) as tc:
        pools = _Pools(tc)

        ps_mm = pools.open("ps_mm", bufs=2, space="PSUM")   # [P,1024] slots (2 banks each)
        ps_st = pools.open("ps_st", bufs=1, space="PSUM")
        ps_pa = pools.open("ps_pa", bufs=1, space="PSUM")
        ps_bc = pools.open("ps_bc", bufs=2, space="PSUM")

        # ---------------- constants / params ----------------
        const = pools.open("const", bufs=1)

        ones_col = const.tile([P, 1], BF16, name="ones_col")
        nc.vector.memset(ones_col[:], 1.0)
        ones_row = const.tile([1, P], FP32, name="ones_row")
        nc.vector.memset(ones_row[:], 1.0)
        ident = const.tile([P, P], FP32, name="ident")
        make_identity(nc, ident[:])

        def load_pf(ap_dram, n_chunks, name):
            tl = const.tile([P, n_chunks], FP32, name=name)
            nc.sync.dma_start(tl[:], ap_dram.rearrange("(c p) -> p c", p=P))
            return tl

        g1 = load_pf(t["ln1_g"], NDC, "g1")
        b1l = load_pf(t["ln1_b"], NDC, "b1l")
        g2 = load_pf(t["ln2_g"], NDC, "g2")
        b2l = load_pf(t["ln2_b"], NDC, "b2l")
        bq_s = load_pf(t["bq"], NDC, "bq_s")
        nc.vector.tensor_scalar_mul(bq_s[:], bq_s[:], 0.125)
        bk_c = load_pf(t["bk"], NDC, "bk_c")
        bo_c = load_pf(t["bo"], NDC, "bo_c")
        b2_c = load_pf(t["b2"], NDC, "b2_c")
        b1_c = load_pf(t["b1"], NFC, "b1_c")
        m01c = load_pf(t["m01"], NKC, "m01c")

        # bv broadcast to all partitions: [1, D] -> [P, D]
        bv_row = const.tile([1, D], FP32, name="bv_row")
        nc.sync.dma_start(bv_row[:], t["bv"].rearrange("(a d) -> a d", a=1))
        bvb = const.tile([P, D], FP32, name="bvb")
        for half in range(2):
            pb = ps_bc.tile([P, 512], FP32, tag="bc")
            nc.tensor.matmul(pb[:], ones_row[:], bv_row[:, ts(half, 512)], start=True, stop=True)
            nc.scalar.copy(bvb[:, ts(half, 512)], pb[:])

        def bcast(row_ap, out_ap, width, parts=P):
            """out[p, :w] = row[0, :w] for p in range(parts), via K=1 matmul."""
            nwhole = width // 512
            for i in range(nwhole):
                pb = ps_bc.tile([P, 512], FP32, tag="bc")
                nc.tensor.matmul(pb[0:parts, :], ones_row[:, 0:parts], row_ap[:, ts(i, 512)],
                                 start=True, stop=True)
                nc.scalar.copy(out_ap[:, ts(i, 512)], pb[0:parts, :])

        # ---------------- phase A: LN1 over TKV tokens ----------------
        pst = pools.open("stsc", bufs=1)
        phq = pools.open("hq", bufs=1)
        ph = pools.open("h", bufs=1)
        pxT = pools.open("xT", bufs=1)
        plnA = pools.open("lnA", bufs=2)

        xT_t = pxT.tile([P, NDC, TKV], FP32, name="xTt")
        xT_r = t["xT"].rearrange("(c p) k -> p c k", p=P)
        for c in range(NDC):
            nc.sync.dma_start(xT_t[:, c, :], xT_r[:, c, :])

        def ln_tile(src3, tok_sl, dst_bf, dst_f32, g, b, spool, tagp):
            """LayerNorm 512 tokens of src3 ([P, NDC, ntok] fp32) at slice
            tok_sl; write bf16 to dst_bf[:, c, tok_sl] and optionally fp32 to
            dst_f32.  ddof=1, denominator (std + eps).  Per-token stats are
            broadcast across partitions first so all scalar math runs on 128
            lanes instead of one."""
            ps = ps_st.tile([33, 512], FP32, tag="st")
            for c in range(NDC):
                cast = spool.tile([P, 1024], BF16, tag=f"cast{tagp}")
                nc.scalar.copy(cast[:, 0:512], src3[:, c, tok_sl])
                nc.tensor.matmul(ps[0:1, :], ones_col[:], cast[:, 0:512],
                                 start=(c == 0), stop=(c == NDC - 1))
                nc.scalar.activation(cast[:, 512:1024], src3[:, c, tok_sl], AF.Square)
                nc.tensor.matmul(ps[32:33, :], ones_col[:], cast[:, 512:1024],
                                 start=(c == 0), stop=(c == NDC - 1))
            srow = pst.tile([1, 512], FP32, tag="srow")
            nc.scalar.copy(srow[:], ps[0:1, :])
            sqrow = pst.tile([1, 512], FP32, tag="sqrow")
            nc.scalar.copy(sqrow[:], ps[32:33, :])
            pbs = ps_bc.tile([P, 512], FP32, tag="bc")
            nc.tensor.matmul(pbs[:], ones_row[:], srow[:], start=True, stop=True)
            pbsq = ps_bc.tile([P, 512], FP32, tag="bc")
            nc.tensor.matmul(pbsq[:], ones_row[:], sqrow[:], start=True, stop=True)
            meanb = spool.tile([P, 512], FP32, tag=f"meanb{tagp}")
            nc.vector.tensor_scalar_mul(meanb[:], pbs[:], 1.0 / D)
            varb = spool.tile([P, 512], FP32, tag=f"varb{tagp}")
            nc.vector.tensor_scalar_mul(varb[:], pbsq[:], 1.0 / (D - 1))
            msqb = spool.tile([P, 512], FP32, tag=f"msqb{tagp}")
            nc.vector.tensor_mul(msqb[:], meanb[:], meanb[:])
            nc.vector.tensor_scalar(msqb[:], msqb[:], float(D) / (D - 1), None, ALU.mult)
            nc.vector.tensor_tensor(varb[:], varb[:], msqb[:], ALU.subtract)
            nc.scalar.activation(varb[:], varb[:], AF.Sqrt)
            nc.vector.tensor_scalar_add(varb[:], varb[:], EPS)
            rinvb = spool.tile([P, 512], FP32, tag=f"rinvb{tagp}")
            nc.vector.reciprocal(rinvb[:], varb[:])
            nmb = spool.tile([P, 512], FP32, tag=f"nmb{tagp}")
            nc.vector.tensor_mul(nmb[:], meanb[:], rinvb[:])
            for c in range(NDC):
                tmp = spool.tile([P, 512], FP32, tag=f"lntmp{tagp}")
                nc.vector.tensor_mul(tmp[:], src3[:, c, tok_sl], rinvb[:])
                nc.vector.tensor_tensor(tmp[:], tmp[:], nmb[:], ALU.subtract)
                nc.vector.tensor_scalar(dst_bf[:, c, tok_sl], tmp[:], g[:, c : c + 1],
                                        b[:, c : c + 1], ALU.mult, ALU.add)
                if dst_f32 is not None:
                    nc.vector.tensor_scalar(dst_f32[:, c, :], tmp[:], g[:, c : c + 1],
                                            b[:, c : c + 1], ALU.mult, ALU.add)

        hT = ph.tile([P, NDC, TKV], BF16, name="hT")
        hq = phq.tile([P, NDC, TQ], FP32, name="hq")
        for tt in range(TKV // 512):
            ln_tile(xT_t, ts(tt, 512), hT, hq if tt == 0 else None, g1, b1l, plnA, "A")
        pools.close("lnA")
        pools.close("xT")

        # ---------------- phase B: Q/K/V projections ----------------
        pw = pools.open("wproj", bufs=2)
        pqkv = pools.open("qkv", bufs=1, side="right")

        wq_t = pw.tile([P, NDC, D], BF16, tag="wm")
        nc.sync.dma_start(wq_t[:], t["wq"].rearrange("(c p) n -> p c n", p=P))
        qT = pqkv.tile([P, NDC, TQ], BF16, name="qT")
        for n in range(NDC):
            ps = ps_mm.tile([P, 512], FP32, tag="mm")
            for c in range(NDC):
                nc.tensor.matmul(ps[:], wq_t[:, c, ts(n, P)], hT[:, c, 0:TQ],
                                 start=(c == 0), stop=(c == NDC - 1))
            nc.vector.tensor_scalar(qT[:, n, :], ps[:], 0.125, bq_s[:, n : n + 1],
                                    ALU.mult, ALU.add)

        wk_t = pw.tile([P, NDC, D], BF16, tag="wm")
        nc.sync.dma_start(wk_t[:], t["wk"].rearrange("(c p) n -> p c n", p=P))
        kT = pqkv.tile([P, NDC, TKV], BF16, name="kT")
        for n in range(NDC):
            for tp in range(TKV // 1024):
                ps = ps_mm.tile([P, 1024], FP32, tag="mm")
                for c in range(NDC):
                    nc.tensor.matmul(ps[:, 0:512], wk_t[:, c, ts(n, P)],
                                     hT[:, c, ts(2 * tp, 512)],
                                     start=(c == 0), stop=(c == NDC - 1))
                    nc.tensor.matmul(ps[:, 512:1024], wk_t[:, c, ts(n, P)],
                                     hT[:, c, ts(2 * tp + 1, 512)],
                                     start=(c == 0), stop=(c == NDC - 1))
                nc.vector.tensor_scalar(kT[:, n, ts(tp, 1024)], ps[:],
                                        bk_c[:, n : n + 1], None, ALU.add)

        wv_t = pw.tile([P, NDC, D], BF16, tag="wm")
        nc.sync.dma_start(wv_t[:], t["wv"].rearrange("(c p) n -> p c n", p=P))
        vaug = pqkv.tile([P, NKC, H, DH + 1], BF16, name="vaug")
        nc.vector.memset(vaug[:, :, :, DH], 1.0)
        for kc in range(NKC):
            ps = ps_mm.tile([P, 1024], FP32, tag="mm")
            for c in range(NDC):
                nc.tensor.matmul(ps[:, 0:512], hT[:, c, ts(kc, P)], wv_t[:, c, 0:512],
                                 start=(c == 0), stop=(c == NDC - 1))
                nc.tensor.matmul(ps[:, 512:1024], hT[:, c, ts(kc, P)], wv_t[:, c, 512:1024],
                                 start=(c == 0), stop=(c == NDC - 1))
            nc.vector.tensor_tensor(
                vaug[:, kc, :, 0:DH],
                ps[:].rearrange("p (h d) -> p h d", d=DH),
                bvb[:].rearrange("p (h d) -> p h d", d=DH),
                ALU.add,
            )
            # multiplicative key mask (exact: scales both V rows and the
            # denominator ones-column; zero for masked keys)
            nc.vector.tensor_scalar(vaug[:, kc, :, :], vaug[:, kc, :, :],
                                    m01c[:, kc : kc + 1], None, ALU.mult)
        pools.close("wproj")
        pools.close("h")

        # wo prefetch overlaps attention (released wproj zone gates the DMA)
        pwo = pools.open("wo", bufs=1)
        wo_t = pwo.tile([P, NDC, D], BF16, name="wo_t")
        nc.sync.dma_start(wo_t[:], t["wo"].rearrange("(c p) n -> p c n", p=P))

        # ---------------- phase C: attention (per head, sw-pipelined) ----------------
        ppt = pools.open("pt", bufs=2, side="right")
        pattn = pools.open("attn", bufs=1)
        attnT = pattn.tile([P, NDC, TQ], BF16, name="attnT")

        pt_tiles = [None] * H

        def emit_scores(h):
            pof = DH * (h % 2)
            ch = h // 2
            pt = ppt.tile([P, NKC, TQ], BF16, tag="pt")
            pt_tiles[h] = pt
            for kp in range(NKC // 2):
                ps = ps_mm.tile([P, 1024], FP32, tag="mm")
                nc.tensor.matmul(ps[:, 0:512], kT[pof : pof + DH, ch, ts(2 * kp, P)],
                                 qT[pof : pof + DH, ch, :], start=True, stop=True)
                nc.tensor.matmul(ps[:, 512:1024], kT[pof : pof + DH, ch, ts(2 * kp + 1, P)],
                                 qT[pof : pof + DH, ch, :], start=True, stop=True)
                nc.scalar.activation(pt[:, 2 * kp : 2 * kp + 2, :],
                                     ps[:].rearrange("p (a b) -> p a b", b=512), AF.Exp)

        def emit_attnv(h):
            pof = DH * (h % 2)
            ch = h // 2
            pt = pt_tiles[h]
            pa = ps_pa.tile([DH + 1, TQ], FP32, tag="pa")
            for kc in range(NKC):
                nc.tensor.matmul(pa[:], vaug[:, kc, h, :], pt[:, kc, :],
                                 start=(kc == 0), stop=(kc == NKC - 1))
            drow = pattn.tile([1, TQ], FP32, tag="drow")
            nc.scalar.copy(drow[:], pa[DH : DH + 1, :])
            pb = ps_bc.tile([P, 512], FP32, tag="bc")
            nc.tensor.matmul(pb[0:DH, :], ones_row[:, 0:DH], drow[:], start=True, stop=True)
            rdb = pattn.tile([DH, TQ], FP32, tag="rdb")
            nc.vector.reciprocal(rdb[:], pb[0:DH, :])
            nc.vector.tensor_mul(attnT[pof : pof + DH, ch, :], pa[0:DH, :], rdb[:])

        # software pipeline: scores(h+1) overlaps exp(h) / attnV(h)
        emit_scores(0)
        for h in range(H):
            if h + 1 < H:
                emit_scores(h + 1)
            emit_attnv(h)
            pt_tiles[h] = None
        pools.close("pt")
        pools.close("qkv")

        # ---------------- phase D: output projection + residual ----------------
        pres = pools.open("res1", bufs=1, side="right")
        res1T = pres.tile([P, NDC, TQ], FP32, name="res1T")
        for n in range(NDC):
            ps = ps_mm.tile([P, 512], FP32, tag="mm")
            for c in range(NDC):
                nc.tensor.matmul(ps[:], wo_t[:, c, ts(n, P)], attnT[:, c, :],
                                 start=(c == 0), stop=(c == NDC - 1))
            nc.vector.tensor_scalar(res1T[:, n, :], ps[:], bo_c[:, n : n + 1], None, ALU.add)
            nc.vector.tensor_add(res1T[:, n, :], res1T[:, n, :], hq[:, n, :])
        pools.close("attn")
        pools.close("wo")
        pools.close("hq")

        # ---------------- phase E: LN2 (TQ tokens) ----------------
        plnE = pools.open("lnE", bufs=2, side="right")
        h2T = plnE.tile([P, NDC, TQ], BF16, name="h2T")
        ln_tile(res1T, slice(0, TQ), h2T, None, g2, b2l, plnE, "E")

        # ---------------- phase F: FFN (interleaved halves) ----------------
        pg1 = pools.open("g1", bufs=1)
        pwf1 = pools.open("wf1", bufs=1)
        pwf2 = pools.open("wf2", bufs=1)
        pout = pools.open("out", bufs=1)

        g1T = pg1.tile([P, NFC, TQ], BF16, name="g1T")
        outT = pout.tile([P, NDC, TQ], FP32, name="outT")
        w1_r = t["w1"].rearrange("(c p) f -> p c f", p=P)
        w2_r = t["w2"].rearrange("(f p) d -> p f d", p=P)

        for hf in range(2):
            w1h = pwf1.tile([P, NDC, DFF // 2], BF16, tag="w1h")
            nc.sync.dma_start(w1h[:], w1_r[:, :, ts(hf, DFF // 2)])
            for fl in range(NFC // 2):
                fc = hf * (NFC // 2) + fl
                ps = ps_mm.tile([P, 512], FP32, tag="mm")
                for c in range(NDC):
                    nc.tensor.matmul(ps[:], w1h[:, c, ts(fl, P)], h2T[:, c, :],
                                     start=(c == 0), stop=(c == NDC - 1))
                nc.scalar.activation(g1T[:, fc, :], ps[:], AF.Relu,
                                     bias=b1_c[:, fc : fc + 1], scale=1.0)
            # second FFN matmul for this half of d_ff
            w2h = pwf2.tile([P, NFC // 2, D], BF16, tag="w2h")
            nc.sync.dma_start(w2h[:], w2_r[:, ts(hf, NFC // 2), :])
            for n in range(NDC):
                ps = ps_mm.tile([P, 512], FP32, tag="mm")
                for fl in range(NFC // 2):
                    fc = hf * (NFC // 2) + fl
                    nc.tensor.matmul(ps[:], w2h[:, fl, ts(n, P)], g1T[:, fc, :],
                                     start=(fl == 0), stop=(fl == NFC // 2 - 1))
                if hf == 0:
                    nc.vector.tensor_scalar(outT[:, n, :], ps[:], b2_c[:, n : n + 1],
                                            None, ALU.add)
                else:
                    nc.vector.tensor_add(outT[:, n, :], outT[:, n, :], ps[:])
        pools.close("lnE")

        # final residual: out = res1 + ffn
        for n in range(NDC):
            nc.vector.tensor_add(outT[:, n, :], outT[:, n, :], res1T[:, n, :])
        pools.close("res1")

        # ---------------- transpose to token-major and store ----------------
        out_sb = pout.tile([P, TQ // P, D], FP32, name="out_sb")
        for n in range(NDC):
            for qc in range(TQ // P):
                pt_ = ps_bc.tile([P, P], FP32, tag="bc")
                nc.tensor.transpose(pt_[:], outT[:, n, ts(qc, P)], ident[:])
                nc.scalar.copy(out_sb[:, qc, ts(n, P)], pt_[:])
        nc.sync.dma_start(t["out"].rearrange("(qc p) d -> p qc d", p=P), out_sb[:])

        pools.close_all()


def _build_nc(split=True):
    nc = bass.Bass("TRN2", target_bir_lowering=False, debug=False)

    t = {}

    def inp(name, shape, dtype=FP32):
        t[name] = nc.dram_tensor(name, shape, dtype, kind="ExternalInput").ap()

    inp("xT", [D, TKV])
    inp("m01", [TKV])
    for nm in ("ln1_g", "ln1_b", "ln2_g", "ln2_b", "bq", "bk", "bv", "bo", "b2"):
        inp(nm, [D])
    inp("b1", [DFF])
    for nm in ("wq", "wk", "wv", "wo"):
        inp(nm, [D, D], BF16)
    inp("w1", [D, DFF], BF16)
    inp("w2", [DFF, D], BF16)
    t["out"] = nc.dram_tensor("out", [TQ, D], FP32, kind="ExternalOutput").ap()

    _emit(nc, t)
    if split:
        _split_multi_waits(nc)
    return nc


_NC_CACHE = None


def _get_nc():
    global _NC_CACHE
    if _NC_CACHE is None:
        _NC_CACHE = _build_nc()
    return _NC_CACHE


def make_in_maps(x, mask, ln1_g, ln1_b, ln2_g, ln2_b,
                 wq, bq, wk, bk, wv, bv, wo, bo, w1, b1, w2, b2):
    """Build the 8 per-core input dicts from the full-size inputs."""
    bf = ml_dtypes.bfloat16
    shared = {
        "ln1_g": np.ascontiguousarray(ln1_g, np.float32),
        "ln1_b": np.ascontiguousarray(ln1_b, np.float32),
        "ln2_g": np.ascontiguousarray(ln2_g, np.float32),
        "ln2_b": np.ascontiguousarray(ln2_b, np.float32),
        "bq": np.ascontiguousarray(bq, np.float32),
        "bk": np.ascontiguousarray(bk, np.float32),
        "bv": np.ascontiguousarray(bv, np.float32),
        "bo": np.ascontiguousarray(bo, np.float32),
        "b1": np.ascontiguousarray(b1, np.float32),
        "b2": np.ascontiguousarray(b2, np.float32),
        "wq": np.ascontiguousarray(np.asarray(wq, np.float32).astype(bf)),
        "wk": np.ascontiguousarray(np.asarray(wk, np.float32).astype(bf)),
        "wv": np.ascontiguousarray(np.asarray(wv, np.float32).astype(bf)),
        "wo": np.ascontiguousarray(np.asarray(wo, np.float32).astype(bf)),
        "w1": np.ascontiguousarray(np.asarray(w1, np.float32).astype(bf)),
        "w2": np.ascontiguousarray(np.asarray(w2, np.float32).astype(bf)),
    }
    x = np.asarray(x, np.float32)
    mask = np.asarray(mask)
    in_maps = []
    for core in range(NCORES):
        b, qb = divmod(core, NCORES // 2)
        shift = -qb * TQ
        xb = np.roll(x[b], shift, axis=0)                      # (TKV, D)
        m01 = np.roll((np.asarray(mask[b]) != 0).astype(np.float32), shift)
        in_maps.append({
            **shared,
            "xT": np.ascontiguousarray(xb.T),
            "m01": np.ascontiguousarray(m01),
        })
    return in_maps


def kernel(**inputs):
    nc = _get_nc()
    in_maps = make_in_maps(**inputs)
    res = run_bass_kernel_spmd(nc, in_maps, core_ids=list(range(NCORES)))
    Bsz, S, _ = np.asarray(inputs["x"]).shape
    out = np.empty((Bsz, S, D), np.float32)
    for core in range(NCORES):
        b, qb = divmod(core, NCORES // 2)
        out[b, qb * TQ : (qb + 1) * TQ, :] = res.results[core]["out"]
    return out


# revision 14
# speedup vs baseline: 1.1220x; 1.0482x over previous
"""Trainium2 Bass kernel for a pre-LN transformer block (B=2, S=2048, D=1024,
H=16 heads, d_ff=4096), data-parallel over 8 NeuronCores.

Sharding: each core owns 512 query tokens (2 batches x 4 blocks).  Every core
LayerNorms the full 2048-token sequence of its batch and computes K/V for all
of it (redundant K/V, zero communication); Q / attention / O-proj / FFN are
computed only for the core's 512 tokens.  The per-core token axis is rolled on
the host so the core's query block always sits at positions 0:512 — attention
is permutation-invariant over keys, so K/V order does not matter as long as
the mask bias is rolled the same way.

On-chip dataflow is entirely feature-major ([d_model, tokens]); LayerNorm
statistics are computed with ones-vector matmuls on the PE, per-token scalars
are broadcast across partitions with K=1 outer-product matmuls, and softmax
denominators come from an extra all-ones column appended to V.  Matmul inputs
are bf16 (weights pre-cast on host), accumulation fp32 in PSUM; residuals,
LayerNorm and softmax math are fp32.
"""

import numpy as np
import ml_dtypes

import concourse.bass as bass
import concourse.mybir as mybir
import concourse.tile as tile
from concourse import bass_utils
from concourse.bass import ts
from concourse.vector_clock import ScopedClock
from concourse.bass_utils import run_bass_kernel_spmd
from concourse.masks import make_identity

AF = mybir.ActivationFunctionType
ALU = mybir.AluOpType
FP32 = mybir.dt.float32
BF16 = mybir.dt.bfloat16

P = 128
D = 1024
H = 16
DH = 64
DFF = 4096
TQ = 512          # query tokens per core
TKV = 2048        # keys/values per core (full batch sequence)
NDC = D // P      # 8 d_model chunks
NFC = DFF // P    # 32 d_ff chunks
NKC = TKV // P    # 16 key chunks
NCORES = 8
EPS = 1e-6

# This container's walrus build accepts at most ONE sync-wait per instruction.
MAXW = 1


class _TileCtx(tile.TileContext):
    """TileContext whose exit drain splits its sem waits across a chain of
    drains (walrus here rejects instructions with >MAXW sync waits)."""

    def _drain_and_barrier(self, tick_clock, wait_clock):
        probe = self.nc.sync.drain()
        wait_clock.add_sem_waits(probe.ins, ScopedClock({None: tick_clock.global_clock}))
        si = probe.ins.sync_info
        waits = list(si.on_wait) if si is not None else []
        if len(waits) > MAXW:
            probe.ins.sync_info = mybir.SyncInfo(
                on_wait=waits[:MAXW], on_update=list(si.on_update)
            )
            for i in range(MAXW, len(waits), MAXW):
                d = self.nc.sync.drain()
                d.ins.sync_info = mybir.SyncInfo(on_wait=waits[i : i + MAXW], on_update=[])
        self.nc.all_engine_barrier()
        assert self.sems is not None
        popped = self.nc._tile_sem_poison_stack.pop()
        assert popped is self._sem_poison
        self.nc.clear_and_free_semaphores(list(self.sems.allocated().values()))
        self.nc.all_engine_barrier()


def _drop_ldweights_prefetch(nc):
    """Remove Tile's standalone InstLdweights prefetches.  The InstMatmults
    are self-loading (carry the stationary operand), and walrus's LDW
    optimization (--enable-ldw-opt=true) refuses standalone InstLdweights.
    Any sem waits on a dropped ldweights move to a same-engine carrier nop."""
    n_drop = 0
    for f in nc.m.functions:
        for blk in f.blocks:
            insts = blk.instructions
            i = 0
            while i < len(insts):
                inst = insts[i]
                if type(inst).__name__ == "InstLdweights":
                    si = getattr(inst, "sync_info", None)
                    if si is not None and (si.on_wait or si.on_update):
                        nop = mybir.InstNoOp(name=f"{inst.name}-ldwdrop", ins=[], outs=[])
                        nop.engine = inst.engine
                        nop.sync_info = mybir.SyncInfo(
                            on_wait=list(si.on_wait), on_update=list(si.on_update)
                        )
                        insts[i] = nop
                        i += 1
                    else:
                        insts.pop(i)
                    n_drop += 1
                else:
                    i += 1
    return n_drop


def _split_multi_waits(nc, maxw=MAXW):
    """Hoist extra sem waits of any instruction onto preceding same-engine
    nops (same queue => in-order => semantically identical)."""
    n_split = 0
    for f in nc.m.functions:
        for blk in f.blocks:
            insts = blk.instructions
            i = 0
            while i < len(insts):
                inst = insts[i]
                si = getattr(inst, "sync_info", None)
                if si is not None and len(si.on_wait) > maxw:
                    waits = list(si.on_wait)
                    extra = waits[maxw:]
                    carriers = []
                    for j in range(0, len(extra), maxw):
                        nop = mybir.InstNoOp(name=f"{inst.name}-wsplit{j}", ins=[], outs=[])
                        nop.engine = inst.engine
                        nop.sync_info = mybir.SyncInfo(on_wait=extra[j : j + maxw], on_update=[])
                        carriers.append(nop)
                    inst.sync_info = mybir.SyncInfo(
                        on_wait=waits[:maxw], on_update=list(si.on_update)
                    )
                    for k, c in enumerate(carriers):
                        insts.insert(i + k, c)
                    i += len(carriers)
                    n_split += 1
                i += 1
    return n_split


class _Pools:
    def __init__(self, tc):
        self.tc = tc
        self._cms = {}
        self._order = []

    def open(self, name, bufs=1, space="SBUF", side=None):
        kw = dict(name=name, bufs=bufs, space=space)
        if side is not None:
            kw["side"] = side
        cm = self.tc.tile_pool(**kw)
        pool = cm.__enter__()
        self._cms[name] = (cm, pool)
        self._order.append(name)
        return pool

    def close(self, name):
        cm, _ = self._cms.pop(name)
        self._order.remove(name)
        cm.__exit__(None, None, None)

    def close_all(self):
        for name in reversed(list(self._order)):
            self.close(name)


def _emit(nc, t):
    with _TileCtx(nc) as tc:
        pools = _Pools(tc)

        ps_mm = pools.open("ps_mm", bufs=2, space="PSUM")   # [P,1024] slots (2 banks each)
        ps_st = pools.open("ps_st", bufs=1, space="PSUM")
        ps_pa = pools.open("ps_pa", bufs=2, space="PSUM")
        ps_bc = pools.open("ps_bc", bufs=1, space="PSUM")

        # ---------------- constants / params ----------------
        const = pools.open("const", bufs=1)

        ones_col = const.tile([P, 1], BF16, name="ones_col")
        nc.vector.memset(ones_col[:], 1.0)
        ones_row = const.tile([1, P], FP32, name="ones_row")
        nc.vector.memset(ones_row[:], 1.0)
        ident = const.tile([P, P], FP32, name="ident")
        make_identity(nc, ident[:])

        def load_pf(ap_dram, n_chunks, name):
            tl = const.tile([P, n_chunks], FP32, name=name)
            nc.sync.dma_start(tl[:], ap_dram.rearrange("(c p) -> p c", p=P))
            return tl

        g1 = load_pf(t["ln1_g"], NDC, "g1")
        b1l = load_pf(t["ln1_b"], NDC, "b1l")
        g2 = load_pf(t["ln2_g"], NDC, "g2")
        b2l = load_pf(t["ln2_b"], NDC, "b2l")
        bq_s = load_pf(t["bq"], NDC, "bq_s")
        nc.vector.tensor_scalar_mul(bq_s[:], bq_s[:], 0.125)
        bk_c = load_pf(t["bk"], NDC, "bk_c")
        bo_c = load_pf(t["bo"], NDC, "bo_c")
        b2_c = load_pf(t["b2"], NDC, "b2_c")
        b1_c = load_pf(t["b1"], NFC, "b1_c")
        m01c = load_pf(t["m01"], NKC, "m01c")

        # bv broadcast to all partitions: [1, D] -> [P, D]
        bv_row = const.tile([1, D], FP32, name="bv_row")
        nc.sync.dma_start(bv_row[:], t["bv"].rearrange("(a d) -> a d", a=1))
        bvb = const.tile([P, D], FP32, name="bvb")
        for half in range(2):
            pb = ps_bc.tile([P, 512], FP32, tag="bc")
            nc.tensor.matmul(pb[:], ones_row[:], bv_row[:, ts(half, 512)], start=True, stop=True)
            nc.scalar.copy(bvb[:, ts(half, 512)], pb[:])

        def bcast(row_ap, out_ap, width, parts=P):
            """out[p, :w] = row[0, :w] for p in range(parts), via K=1 matmul."""
            nwhole = width // 512
            for i in range(nwhole):
                pb = ps_bc.tile([P, 512], FP32, tag="bc")
                nc.tensor.matmul(pb[0:parts, :], ones_row[:, 0:parts], row_ap[:, ts(i, 512)],
                                 start=True, stop=True)
                nc.scalar.copy(out_ap[:, ts(i, 512)], pb[0:parts, :])

        # ---------------- phase A: LN1 over TKV tokens ----------------
        pst = pools.open("stsc", bufs=1)
        phq = pools.open("hq", bufs=1)
        ph = pools.open("h", bufs=1)
        pxT = pools.open("xT", bufs=1)
        plnA = pools.open("lnA", bufs=2)

        xT_t = pxT.tile([P, NDC, TKV], FP32, name="xTt")
        xT_r = t["xT"].rearrange("(c p) k -> p c k", p=P)
        for c in range(NDC):
            nc.sync.dma_start(xT_t[:, c, :], xT_r[:, c, :])

        def ln_tile(src3, tok_sl, dst_bf, dst_f32, g, b, spool, tagp):
            """LayerNorm 512 tokens of src3 ([P, NDC, ntok] fp32) at slice
            tok_sl; write bf16 to dst_bf[:, c, tok_sl] and optionally fp32 to
            dst_f32.  ddof=1, denominator (std + eps).  Per-token stats are
            broadcast across partitions first so all scalar math runs on 128
            lanes instead of one."""
            ps = ps_st.tile([33, 512], FP32, tag="st")
            for c in range(NDC):
                cast = spool.tile([P, 1024], BF16, tag=f"cast{tagp}")
                nc.scalar.copy(cast[:, 0:512], src3[:, c, tok_sl])
                nc.tensor.matmul(ps[0:1, :], ones_col[:], cast[:, 0:512],
                                 start=(c == 0), stop=(c == NDC - 1))
                nc.scalar.activation(cast[:, 512:1024], src3[:, c, tok_sl], AF.Square)
                nc.tensor.matmul(ps[32:33, :], ones_col[:], cast[:, 512:1024],
                                 start=(c == 0), stop=(c == NDC - 1))
            srow = pst.tile([1, 512], FP32, tag="srow")
            nc.scalar.copy(srow[:], ps[0:1, :])
            sqrow = pst.tile([1, 512], FP32, tag="sqrow")
            nc.scalar.copy(sqrow[:], ps[32:33, :])
            pbs = ps_bc.tile([P, 512], FP32, tag="bc")
            nc.tensor.matmul(pbs[:], ones_row[:], srow[:], start=True, stop=True)
            pbsq = ps_bc.tile([P, 512], FP32, tag="bc")
            nc.tensor.matmul(pbsq[:], ones_row[:], sqrow[:], start=True, stop=True)
            meanb = spool.tile([P, 512], FP32, tag=f"meanb{tagp}")
            nc.vector.tensor_scalar_mul(meanb[:], pbs[:], 1.0 / D)
            varb = spool.tile([P, 512], FP32, tag=f"varb{tagp}")
            nc.vector.tensor_scalar_mul(varb[:], pbsq[:], 1.0 / (D - 1))
            msqb = spool.tile([P, 512], FP32, tag=f"msqb{tagp}")
            nc.vector.tensor_mul(msqb[:], meanb[:], meanb[:])
            nc.vector.tensor_scalar(msqb[:], msqb[:], float(D) / (D - 1), None, ALU.mult)
            nc.vector.tensor_tensor(varb[:], varb[:], msqb[:], ALU.subtract)
            nc.scalar.activation(varb[:], varb[:], AF.Sqrt)
            nc.vector.tensor_scalar_add(varb[:], varb[:], EPS)
            rinvb = spool.tile([P, 512], FP32, tag=f"rinvb{tagp}")
            nc.vector.reciprocal(rinvb[:], varb[:])
            nmb = spool.tile([P, 512], FP32, tag=f"nmb{tagp}")
            nc.vector.tensor_mul(nmb[:], meanb[:], rinvb[:])
            for c in range(NDC):
                tmp = spool.tile([P, 512], FP32, tag=f"lntmp{tagp}")
                nc.vector.tensor_mul(tmp[:], src3[:, c, tok_sl], rinvb[:])
                nc.vector.tensor_tensor(tmp[:], tmp[:], nmb[:], ALU.subtract)
                nc.vector.tensor_scalar(dst_bf[:, c, tok_sl], tmp[:], g[:, c : c + 1],
                                        b[:, c : c + 1], ALU.mult, ALU.add)
                if dst_f32 is not None:
                    nc.vector.tensor_scalar(dst_f32[:, c, :], tmp[:], g[:, c : c + 1],
                                            b[:, c : c + 1], ALU.mult, ALU.add)

        hT = ph.tile([P, NDC, TKV], BF16, name="hT")
        hq = phq.tile([P, NDC, TQ], FP32, name="hq")
        for tt in range(TKV // 512):
            ln_tile(xT_t, ts(tt, 512), hT, hq if tt == 0 else None, g1, b1l, plnA, "A")
        pools.close("lnA")
        pools.close("xT")

        # ---------------- phase B: Q/K/V projections ----------------
        pw = pools.open("wproj", bufs=2)
        pqkv = pools.open("qkv", bufs=1, side="right")

        wq_t = pw.tile([P, NDC, D], BF16, tag="wm")
        nc.sync.dma_start(wq_t[:], t["wq"].rearrange("(c p) n -> p c n", p=P))
        qT = pqkv.tile([P, NDC, TQ], BF16, name="qT")
        for n in range(NDC):
            ps = ps_mm.tile([P, 512], FP32, tag="mm")
            for c in range(NDC):
                nc.tensor.matmul(ps[:], wq_t[:, c, ts(n, P)], hT[:, c, 0:TQ],
                                 start=(c == 0), stop=(c == NDC - 1))
            nc.vector.tensor_scalar(qT[:, n, :], ps[:], 0.125, bq_s[:, n : n + 1],
                                    ALU.mult, ALU.add)

        wk_t = pw.tile([P, NDC, D], BF16, tag="wm")
        nc.sync.dma_start(wk_t[:], t["wk"].rearrange("(c p) n -> p c n", p=P))
        kT = pqkv.tile([P, NDC, TKV], BF16, name="kT")
        for n in range(NDC):
            for tp in range(TKV // 1024):
                ps = ps_mm.tile([P, 1024], FP32, tag="mm")
                for c in range(NDC):
                    nc.tensor.matmul(ps[:, 0:512], wk_t[:, c, ts(n, P)],
                                     hT[:, c, ts(2 * tp, 512)],
                                     start=(c == 0), stop=(c == NDC - 1))
                    nc.tensor.matmul(ps[:, 512:1024], wk_t[:, c, ts(n, P)],
                                     hT[:, c, ts(2 * tp + 1, 512)],
                                     start=(c == 0), stop=(c == NDC - 1))
                nc.vector.tensor_scalar(kT[:, n, ts(tp, 1024)], ps[:],
                                        bk_c[:, n : n + 1], None, ALU.add)

        wv_t = pw.tile([P, NDC, D], BF16, tag="wm")
        nc.sync.dma_start(wv_t[:], t["wv"].rearrange("(c p) n -> p c n", p=P))
        vaug = pqkv.tile([P, NKC, H, DH + 1], BF16, name="vaug")
        nc.vector.memset(vaug[:, :, :, DH], 1.0)
        for kc in range(NKC):
            ps = ps_mm.tile([P, 1024], FP32, tag="mm")
            for c in range(NDC):
                nc.tensor.matmul(ps[:, 0:512], hT[:, c, ts(kc, P)], wv_t[:, c, 0:512],
                                 start=(c == 0), stop=(c == NDC - 1))
                nc.tensor.matmul(ps[:, 512:1024], hT[:, c, ts(kc, P)], wv_t[:, c, 512:1024],
                                 start=(c == 0), stop=(c == NDC - 1))
            nc.vector.tensor_tensor(
                vaug[:, kc, :, 0:DH],
                ps[:].rearrange("p (h d) -> p h d", d=DH),
                bvb[:].rearrange("p (h d) -> p h d", d=DH),
                ALU.add,
            )
            # multiplicative key mask (exact: scales both V rows and the
            # denominator ones-column; zero for masked keys)
            nc.vector.tensor_scalar(vaug[:, kc, :, :], vaug[:, kc, :, :],
                                    m01c[:, kc : kc + 1], None, ALU.mult)
        pools.close("wproj")
        pools.close("h")

        # wo prefetch overlaps attention (released wproj zone gates the DMA)
        pwo = pools.open("wo", bufs=1)
        wo_t = pwo.tile([P, NDC, D], BF16, name="wo_t")
        nc.sync.dma_start(wo_t[:], t["wo"].rearrange("(c p) n -> p c n", p=P))

        # ---------------- phase C: attention (per head, sw-pipelined) ----------------
        ppt = pools.open("pt", bufs=2, side="right")
        pattn = pools.open("attn", bufs=1)
        attnT = pattn.tile([P, NDC, TQ], BF16, name="attnT")

        pt_tiles = [None] * H

        def emit_scores(h):
            pof = DH * (h % 2)
            ch = h // 2
            pt = ppt.tile([P, NKC, TQ], BF16, tag="pt")
            pt_tiles[h] = pt
            for kp in range(NKC // 2):
                ps = ps_mm.tile([P, 1024], FP32, tag="mm")
                nc.tensor.matmul(ps[:, 0:512], kT[pof : pof + DH, ch, ts(2 * kp, P)],
                                 qT[pof : pof + DH, ch, :], start=True, stop=True)
                nc.tensor.matmul(ps[:, 512:1024], kT[pof : pof + DH, ch, ts(2 * kp + 1, P)],
                                 qT[pof : pof + DH, ch, :], start=True, stop=True)
                nc.scalar.activation(pt[:, 2 * kp : 2 * kp + 2, :],
                                     ps[:].rearrange("p (a b) -> p a b", b=512), AF.Exp)

        def emit_attnv(h):
            pof = DH * (h % 2)
            ch = h // 2
            pt = pt_tiles[h]
            pa = ps_pa.tile([DH + 1, TQ], FP32, tag="pa")
            for kc in range(NKC):
                nc.tensor.matmul(pa[:], vaug[:, kc, h, :], pt[:, kc, :],
                                 start=(kc == 0), stop=(kc == NKC - 1))
            drow = pattn.tile([1, TQ], FP32, tag="drow")
            nc.vector.tensor_copy(drow[:], pa[DH : DH + 1, :])
            pb = ps_bc.tile([P, 512], FP32, tag="bc")
            nc.tensor.matmul(pb[0:DH, :], ones_row[:, 0:DH], drow[:], start=True, stop=True)
            rdb = pattn.tile([DH, TQ], FP32, tag="rdb")
            nc.vector.reciprocal(rdb[:], pb[0:DH, :])
            nc.vector.tensor_mul(attnT[pof : pof + DH, ch, :], pa[0:DH, :], rdb[:])

        # software pipeline: scores(h+1) overlaps exp(h) / attnV(h)
        emit_scores(0)
        for h in range(H):
            if h + 1 < H:
                emit_scores(h + 1)
            emit_attnv(h)
            pt_tiles[h] = None
        pools.close("pt")
        pools.close("qkv")

        # ---------------- phase D: output projection + residual ----------------
        pres = pools.open("res1", bufs=1, side="right")
        res1T = pres.tile([P, NDC, TQ], FP32, name="res1T")
        for n in range(NDC):
            ps = ps_mm.tile([P, 512], FP32, tag="mm")
            for c in range(NDC):
                nc.tensor.matmul(ps[:], wo_t[:, c, ts(n, P)], attnT[:, c, :],
                                 start=(c == 0), stop=(c == NDC - 1))
            nc.vector.tensor_scalar(res1T[:, n, :], ps[:], bo_c[:, n : n + 1], None, ALU.add)
            nc.vector.tensor_add(res1T[:, n, :], res1T[:, n, :], hq[:, n, :])
        pools.close("attn")
        pools.close("wo")
        pools.close("hq")

        # ---------------- phase E: LN2 (TQ tokens) ----------------
        plnE = pools.open("lnE", bufs=2, side="right")
        h2T = plnE.tile([P, NDC, TQ], BF16, name="h2T")
        ln_tile(res1T, slice(0, TQ), h2T, None, g2, b2l, plnE, "E")

        # ---------------- phase F: FFN (interleaved halves) ----------------
        pg1 = pools.open("g1", bufs=1)
        pwf1 = pools.open("wf1", bufs=1)
        pwf2 = pools.open("wf2", bufs=1)
        pout = pools.open("out", bufs=1)

        g1T = pg1.tile([P, NFC, TQ], BF16, name="g1T")
        outT = pout.tile([P, NDC, TQ], FP32, name="outT")
        w1_r = t["w1"].rearrange("(c p) f -> p c f", p=P)
        w2_r = t["w2"].rearrange("(f p) d -> p f d", p=P)

        for hf in range(2):
            w1h = pwf1.tile([P, NDC, DFF // 2], BF16, tag="w1h")
            nc.sync.dma_start(w1h[:], w1_r[:, :, ts(hf, DFF // 2)])
            for fl in range(NFC // 2):
                fc = hf * (NFC // 2) + fl
                ps = ps_mm.tile([P, 512], FP32, tag="mm")
                for c in range(NDC):
                    nc.tensor.matmul(ps[:], w1h[:, c, ts(fl, P)], h2T[:, c, :],
                                     start=(c == 0), stop=(c == NDC - 1))
                nc.scalar.activation(g1T[:, fc, :], ps[:], AF.Relu,
                                     bias=b1_c[:, fc : fc + 1], scale=1.0)
            # second FFN matmul for this half of d_ff
            w2h = pwf2.tile([P, NFC // 2, D], BF16, tag="w2h")
            nc.sync.dma_start(w2h[:], w2_r[:, ts(hf, NFC // 2), :])
            for n in range(NDC):
                ps = ps_mm.tile([P, 512], FP32, tag="mm")
                for fl in range(NFC // 2):
                    fc = hf * (NFC // 2) + fl
                    nc.tensor.matmul(ps[:], w2h[:, fl, ts(n, P)], g1T[:, fc, :],
                                     start=(fl == 0), stop=(fl == NFC // 2 - 1))
                if hf == 0:
                    nc.vector.tensor_scalar(outT[:, n, :], ps[:], b2_c[:, n : n + 1],
                                            None, ALU.add)
                else:
                    nc.vector.tensor_add(outT[:, n, :], outT[:, n, :], ps[:])
        pools.close("lnE")

        # final residual: out = res1 + ffn
        for n in range(NDC):
            nc.vector.tensor_add(outT[:, n, :], outT[:, n, :], res1T[:, n, :])
        pools.close("res1")

        # ---------------- transpose to token-major and store ----------------
        out_sb = pout.tile([P, TQ // P, D], FP32, name="out_sb")
        for n in range(NDC):
            for qc in range(TQ // P):
                pt_ = ps_bc.tile([P, P], FP32, tag="bc")
                nc.tensor.transpose(pt_[:], outT[:, n, ts(qc, P)], ident[:])
                nc.scalar.copy(out_sb[:, qc, ts(n, P)], pt_[:])
        nc.sync.dma_start(t["out"].rearrange("(qc p) d -> p qc d", p=P), out_sb[:])

        pools.close_all()


def _build_nc(split=True):
    nc = bass.Bass("TRN2", target_bir_lowering=False, debug=False)

    t = {}

    def inp(name, shape, dtype=FP32):
        t[name] = nc.dram_tensor(name, shape, dtype, kind="ExternalInput").ap()

    inp("xT", [D, TKV])
    inp("m01", [TKV])
    for nm in ("ln1_g", "ln1_b", "ln2_g", "ln2_b", "bq", "bk", "bv", "bo", "b2"):
        inp(nm, [D])
    inp("b1", [DFF])
    for nm in ("wq", "wk", "wv", "wo"):
        inp(nm, [D, D], BF16)
    inp("w1", [D, DFF], BF16)
    inp("w2", [DFF, D], BF16)
    t["out"] = nc.dram_tensor("out", [TQ, D], FP32, kind="ExternalOutput").ap()

    _emit(nc, t)
    if split:
        _split_multi_waits(nc)
    return nc


_NC_CACHE = None


def _get_nc():
    global _NC_CACHE
    if _NC_CACHE is None:
        _NC_CACHE = _build_nc()
    return _NC_CACHE


def make_in_maps(x, mask, ln1_g, ln1_b, ln2_g, ln2_b,
                 wq, bq, wk, bk, wv, bv, wo, bo, w1, b1, w2, b2):
    """Build the 8 per-core input dicts from the full-size inputs."""
    bf = ml_dtypes.bfloat16
    shared = {
        "ln1_g": np.ascontiguousarray(ln1_g, np.float32),
        "ln1_b": np.ascontiguousarray(ln1_b, np.float32),
        "ln2_g": np.ascontiguousarray(ln2_g, np.float32),
        "ln2_b": np.ascontiguousarray(ln2_b, np.float32),
        "bq": np.ascontiguousarray(bq, np.float32),
        "bk": np.ascontiguousarray(bk, np.float32),
        "bv": np.ascontiguousarray(bv, np.float32),
        "bo": np.ascontiguousarray(bo, np.float32),
        "b1": np.ascontiguousarray(b1, np.float32),
        "b2": np.ascontiguousarray(b2, np.float32),
        "wq": np.ascontiguousarray(np.asarray(wq, np.float32).astype(bf)),
        "wk": np.ascontiguousarray(np.asarray(wk, np.float32).astype(bf)),
        "wv": np.ascontiguousarray(np.asarray(wv, np.float32).astype(bf)),
        "wo": np.ascontiguousarray(np.asarray(wo, np.float32).astype(bf)),
        "w1": np.ascontiguousarray(np.asarray(w1, np.float32).astype(bf)),
        "w2": np.ascontiguousarray(np.asarray(w2, np.float32).astype(bf)),
    }
    x = np.asarray(x, np.float32)
    mask = np.asarray(mask)
    in_maps = []
    for core in range(NCORES):
        b, qb = divmod(core, NCORES // 2)
        shift = -qb * TQ
        xb = np.roll(x[b], shift, axis=0)                      # (TKV, D)
        m01 = np.roll((np.asarray(mask[b]) != 0).astype(np.float32), shift)
        in_maps.append({
            **shared,
            "xT": np.ascontiguousarray(xb.T),
            "m01": np.ascontiguousarray(m01),
        })
    return in_maps


def kernel(**inputs):
    nc = _get_nc()
    in_maps = make_in_maps(**inputs)
    res = run_bass_kernel_spmd(nc, in_maps, core_ids=list(range(NCORES)))
    Bsz, S, _ = np.asarray(inputs["x"]).shape
    out = np.empty((Bsz, S, D), np.float32)
    for core in range(NCORES):
        b, qb = divmod(core, NCORES // 2)
        out[b, qb * TQ : (qb + 1) * TQ, :] = res.results[core]["out"]
    return out


# revision 17
# speedup vs baseline: 1.1862x; 1.0573x over previous
"""Trainium2 Bass kernel for a pre-LN transformer block (B=2, S=2048, D=1024,
H=16 heads, d_ff=4096), data-parallel over 8 NeuronCores.

Sharding: each core owns 512 tokens (2 batches x 4 blocks).  Every core
LayerNorms and projects Q/K/V for its own 512 tokens only; K and V (with the
key mask and the softmax-denominator ones-column folded in) are exchanged
between the 4 cores of each batch with a single AllGather, then each core runs
attention for its 512 queries over the full 2048 keys, followed by O-proj,
LN2 and the FFN on its own tokens.

On-chip dataflow is entirely feature-major ([d_model, tokens]); LayerNorm
statistics are computed with ones-vector matmuls on the PE, per-token scalars
are broadcast across partitions with K=1 outer-product matmuls, and softmax
denominators come from an extra all-ones column appended to V.  Matmul inputs
are bf16 (weights pre-cast on host), accumulation fp32 in PSUM; residuals,
LayerNorm and softmax math are fp32.
"""

import numpy as np
import ml_dtypes

import concourse.bass as bass
import concourse.mybir as mybir
import concourse.tile as tile
from concourse import bass_utils
from concourse.bass import ts
from concourse.vector_clock import ScopedClock
from concourse.bass_utils import run_bass_kernel_spmd
from concourse.masks import make_identity

AF = mybir.ActivationFunctionType
ALU = mybir.AluOpType
FP32 = mybir.dt.float32
BF16 = mybir.dt.bfloat16

P = 128
D = 1024
H = 16
DH = 64
DFF = 4096
TQ = 512          # query tokens per core
TKV = 2048        # keys/values per core (full batch sequence)
NDC = D // P      # 8 d_model chunks
NFC = DFF // P    # 32 d_ff chunks
NKC = TKV // P    # 16 key chunks
NCORES = 8
EPS = 1e-6

# This container's walrus build accepts at most ONE sync-wait per instruction.
MAXW = 1


class _TileCtx(tile.TileContext):
    """TileContext whose exit drain splits its sem waits across a chain of
    drains (walrus here rejects instructions with >MAXW sync waits)."""

    def _drain_and_barrier(self, tick_clock, wait_clock):
        probe = self.nc.sync.drain()
        wait_clock.add_sem_waits(probe.ins, ScopedClock({None: tick_clock.global_clock}))
        si = probe.ins.sync_info
        waits = list(si.on_wait) if si is not None else []
        if len(waits) > MAXW:
            probe.ins.sync_info = mybir.SyncInfo(
                on_wait=waits[:MAXW], on_update=list(si.on_update)
            )
            for i in range(MAXW, len(waits), MAXW):
                d = self.nc.sync.drain()
                d.ins.sync_info = mybir.SyncInfo(on_wait=waits[i : i + MAXW], on_update=[])
        self.nc.all_engine_barrier()
        assert self.sems is not None
        popped = self.nc._tile_sem_poison_stack.pop()
        assert popped is self._sem_poison
        self.nc.clear_and_free_semaphores(list(self.sems.allocated().values()))
        self.nc.all_engine_barrier()


def _drop_ldweights_prefetch(nc):
    """Remove Tile's standalone InstLdweights prefetches.  The InstMatmults
    are self-loading (carry the stationary operand), and walrus's LDW
    optimization (--enable-ldw-opt=true) refuses standalone InstLdweights.
    Any sem waits on a dropped ldweights move to a same-engine carrier nop."""
    n_drop = 0
    for f in nc.m.functions:
        for blk in f.blocks:
            insts = blk.instructions
            i = 0
            while i < len(insts):
                inst = insts[i]
                if type(inst).__name__ == "InstLdweights":
                    si = getattr(inst, "sync_info", None)
                    if si is not None and (si.on_wait or si.on_update):
                        nop = mybir.InstNoOp(name=f"{inst.name}-ldwdrop", ins=[], outs=[])
                        nop.engine = inst.engine
                        nop.sync_info = mybir.SyncInfo(
                            on_wait=list(si.on_wait), on_update=list(si.on_update)
                        )
                        insts[i] = nop
                        i += 1
                    else:
                        insts.pop(i)
                    n_drop += 1
                else:
                    i += 1
    return n_drop


def _split_multi_waits(nc, maxw=MAXW):
    """Hoist extra sem waits of any instruction onto preceding same-engine
    nops (same queue => in-order => semantically identical)."""
    n_split = 0
    for f in nc.m.functions:
        for blk in f.blocks:
            insts = blk.instructions
            i = 0
            while i < len(insts):
                inst = insts[i]
                si = getattr(inst, "sync_info", None)
                if si is not None and len(si.on_wait) > maxw:
                    waits = list(si.on_wait)
                    extra = waits[maxw:]
                    carriers = []
                    for j in range(0, len(extra), maxw):
                        nop = mybir.InstNoOp(name=f"{inst.name}-wsplit{j}", ins=[], outs=[])
                        nop.engine = inst.engine
                        nop.sync_info = mybir.SyncInfo(on_wait=extra[j : j + maxw], on_update=[])
                        carriers.append(nop)
                    inst.sync_info = mybir.SyncInfo(
                        on_wait=waits[:maxw], on_update=list(si.on_update)
                    )
                    for k, c in enumerate(carriers):
                        insts.insert(i + k, c)
                    i += len(carriers)
                    n_split += 1
                i += 1
    return n_split


class _Pools:
    def __init__(self, tc):
        self.tc = tc
        self._cms = {}
        self._order = []

    def open(self, name, bufs=1, space="SBUF", side=None):
        kw = dict(name=name, bufs=bufs, space=space)
        if side is not None:
            kw["side"] = side
        cm = self.tc.tile_pool(**kw)
        pool = cm.__enter__()
        self._cms[name] = (cm, pool)
        self._order.append(name)
        return pool

    def close(self, name):
        cm, _ = self._cms.pop(name)
        self._order.remove(name)
        cm.__exit__(None, None, None)

    def close_all(self):
        for name in reversed(list(self._order)):
            self.close(name)


def _emit(nc, t):
    with _TileCtx(nc) as tc:
        pools = _Pools(tc)

        ps_mm = pools.open("ps_mm", bufs=2, space="PSUM")   # [P,1024] slots (2 banks each)
        ps_st = pools.open("ps_st", bufs=1, space="PSUM")
        ps_pa = pools.open("ps_pa", bufs=2, space="PSUM")
        ps_bc = pools.open("ps_bc", bufs=1, space="PSUM")

        # ---------------- constants / params ----------------
        const = pools.open("const", bufs=1)

        ones_col = const.tile([P, 1], BF16, name="ones_col")
        nc.vector.memset(ones_col[:], 1.0)
        ones_row = const.tile([1, P], FP32, name="ones_row")
        nc.vector.memset(ones_row[:], 1.0)
        ident = const.tile([P, P], FP32, name="ident")
        make_identity(nc, ident[:])

        def load_pf(ap_dram, n_chunks, name):
            tl = const.tile([P, n_chunks], FP32, name=name)
            nc.sync.dma_start(tl[:], ap_dram.rearrange("(c p) -> p c", p=P))
            return tl

        g1 = load_pf(t["ln1_g"], NDC, "g1")
        b1l = load_pf(t["ln1_b"], NDC, "b1l")
        g2 = load_pf(t["ln2_g"], NDC, "g2")
        b2l = load_pf(t["ln2_b"], NDC, "b2l")
        bq_s = load_pf(t["bq"], NDC, "bq_s")
        nc.vector.tensor_scalar_mul(bq_s[:], bq_s[:], 0.125)
        bk_c = load_pf(t["bk"], NDC, "bk_c")
        bo_c = load_pf(t["bo"], NDC, "bo_c")
        b2_c = load_pf(t["b2"], NDC, "b2_c")
        b1_c = load_pf(t["b1"], NFC, "b1_c")
        m01c = load_pf(t["m01"], TQ // P, "m01c")

        # bv broadcast to all partitions: [1, D] -> [P, D]
        bv_row = const.tile([1, D], FP32, name="bv_row")
        nc.sync.dma_start(bv_row[:], t["bv"].rearrange("(a d) -> a d", a=1))
        bvb = const.tile([P, D], FP32, name="bvb")
        for half in range(2):
            pb = ps_bc.tile([P, 512], FP32, tag="bc")
            nc.tensor.matmul(pb[:], ones_row[:], bv_row[:, ts(half, 512)], start=True, stop=True)
            nc.scalar.copy(bvb[:, ts(half, 512)], pb[:])

        def bcast(row_ap, out_ap, width, parts=P):
            """out[p, :w] = row[0, :w] for p in range(parts), via K=1 matmul."""
            nwhole = width // 512
            for i in range(nwhole):
                pb = ps_bc.tile([P, 512], FP32, tag="bc")
                nc.tensor.matmul(pb[0:parts, :], ones_row[:, 0:parts], row_ap[:, ts(i, 512)],
                                 start=True, stop=True)
                nc.scalar.copy(out_ap[:, ts(i, 512)], pb[0:parts, :])

        # ---------------- phase A: LN1 over own TQ tokens ----------------
        pst = pools.open("stsc", bufs=1)
        phq = pools.open("hq", bufs=1)
        ph = pools.open("h", bufs=1)
        pxT = pools.open("xT", bufs=1)
        plnA = pools.open("lnA", bufs=2)

        xT_t = pxT.tile([P, NDC, TQ], FP32, name="xTt")
        xT_r = t["xT"].rearrange("(c p) k -> p c k", p=P)
        for c in range(NDC):
            nc.sync.dma_start(xT_t[:, c, :], xT_r[:, c, :])

        def ln_tile(src3, tok_sl, dst_bf, dst_f32, g, b, spool, tagp):
            """LayerNorm 512 tokens of src3 ([P, NDC, ntok] fp32) at slice
            tok_sl; write bf16 to dst_bf[:, c, tok_sl] and optionally fp32 to
            dst_f32.  ddof=1, denominator (std + eps).  Per-token stats are
            broadcast across partitions first so all scalar math runs on 128
            lanes instead of one."""
            ps = ps_st.tile([33, 512], FP32, tag="st")
            for c in range(NDC):
                cast = spool.tile([P, 1024], BF16, tag=f"cast{tagp}")
                nc.scalar.copy(cast[:, 0:512], src3[:, c, tok_sl])
                nc.tensor.matmul(ps[0:1, :], ones_col[:], cast[:, 0:512],
                                 start=(c == 0), stop=(c == NDC - 1))
                nc.scalar.activation(cast[:, 512:1024], src3[:, c, tok_sl], AF.Square)
                nc.tensor.matmul(ps[32:33, :], ones_col[:], cast[:, 512:1024],
                                 start=(c == 0), stop=(c == NDC - 1))
            srow = pst.tile([1, 512], FP32, tag="srow")
            nc.scalar.copy(srow[:], ps[0:1, :])
            sqrow = pst.tile([1, 512], FP32, tag="sqrow")
            nc.scalar.copy(sqrow[:], ps[32:33, :])
            pbs = ps_bc.tile([P, 512], FP32, tag="bc")
            nc.tensor.matmul(pbs[:], ones_row[:], srow[:], start=True, stop=True)
            pbsq = ps_bc.tile([P, 512], FP32, tag="bc")
            nc.tensor.matmul(pbsq[:], ones_row[:], sqrow[:], start=True, stop=True)
            meanb = spool.tile([P, 512], FP32, tag=f"meanb{tagp}")
            nc.vector.tensor_scalar_mul(meanb[:], pbs[:], 1.0 / D)
            varb = spool.tile([P, 512], FP32, tag=f"varb{tagp}")
            nc.vector.tensor_scalar_mul(varb[:], pbsq[:], 1.0 / (D - 1))
            msqb = spool.tile([P, 512], FP32, tag=f"msqb{tagp}")
            nc.vector.tensor_mul(msqb[:], meanb[:], meanb[:])
            nc.vector.tensor_scalar(msqb[:], msqb[:], float(D) / (D - 1), None, ALU.mult)
            nc.vector.tensor_tensor(varb[:], varb[:], msqb[:], ALU.subtract)
            nc.scalar.activation(varb[:], varb[:], AF.Sqrt)
            nc.vector.tensor_scalar_add(varb[:], varb[:], EPS)
            rinvb = spool.tile([P, 512], FP32, tag=f"rinvb{tagp}")
            nc.vector.reciprocal(rinvb[:], varb[:])
            nmb = spool.tile([P, 512], FP32, tag=f"nmb{tagp}")
            nc.vector.tensor_mul(nmb[:], meanb[:], rinvb[:])
            for c in range(NDC):
                tmp = spool.tile([P, 512], FP32, tag=f"lntmp{tagp}")
                nc.vector.tensor_mul(tmp[:], src3[:, c, tok_sl], rinvb[:])
                nc.vector.tensor_tensor(tmp[:], tmp[:], nmb[:], ALU.subtract)
                nc.vector.tensor_scalar(dst_bf[:, c, tok_sl], tmp[:], g[:, c : c + 1],
                                        b[:, c : c + 1], ALU.mult, ALU.add)
                if dst_f32 is not None:
                    nc.vector.tensor_scalar(dst_f32[:, c, :], tmp[:], g[:, c : c + 1],
                                            b[:, c : c + 1], ALU.mult, ALU.add)

        hT = ph.tile([P, NDC, TQ], BF16, name="hT")
        hq = phq.tile([P, NDC, TQ], FP32, name="hq")
        ln_tile(xT_t, slice(0, TQ), hT, hq, g1, b1l, plnA, "A")
        pools.close("lnA")
        pools.close("xT")

        # ---------------- phase B: local Q/K/V projections + AllGather ----------------
        KBLK = D * TQ
        VBLK = TQ * H * (DH + 1)
        pw = pools.open("wproj", bufs=2)
        pqkv = pools.open("qkv", bufs=1, side="right")

        # K local: feature-major [D, TQ], written to the AG input buffer
        wk_t = pw.tile([P, NDC, D], BF16, tag="wm")
        nc.sync.dma_start(wk_t[:], t["wk"].rearrange("(c p) n -> p c n", p=P))
        kT_loc = pqkv.tile([P, NDC, TQ], BF16, name="kT_loc")
        for n in range(NDC):
            ps = ps_mm.tile([P, 512], FP32, tag="mm")
            for c in range(NDC):
                nc.tensor.matmul(ps[:], wk_t[:, c, ts(n, P)], hT[:, c, :],
                                 start=(c == 0), stop=(c == NDC - 1))
            nc.vector.tensor_scalar(kT_loc[:, n, :], ps[:], bk_c[:, n : n + 1], None, ALU.add)
        agk_in = t["ag_in"][0:KBLK].rearrange("(c p q) -> p c q", p=P, q=TQ)
        nc.sync.dma_start(agk_in, kT_loc[:])

        # V local (token-major, heads x (DH+1) with ones column, bias and key
        # mask folded in), written to the AG input buffer
        wv_t = pw.tile([P, NDC, D], BF16, tag="wm")
        nc.sync.dma_start(wv_t[:], t["wv"].rearrange("(c p) n -> p c n", p=P))
        vloc = pqkv.tile([P, TQ // P, H, DH + 1], BF16, name="vloc")
        nc.vector.memset(vloc[:, :, :, DH], 1.0)
        for j in range(TQ // P):
            ps = ps_mm.tile([P, 1024], FP32, tag="mm")
            for c in range(NDC):
                nc.tensor.matmul(ps[:, 0:512], hT[:, c, ts(j, P)], wv_t[:, c, 0:512],
                                 start=(c == 0), stop=(c == NDC - 1))
                nc.tensor.matmul(ps[:, 512:1024], hT[:, c, ts(j, P)], wv_t[:, c, 512:1024],
                                 start=(c == 0), stop=(c == NDC - 1))
            nc.vector.tensor_tensor(
                vloc[:, j, :, 0:DH],
                ps[:].rearrange("p (h d) -> p h d", d=DH),
                bvb[:].rearrange("p (h d) -> p h d", d=DH),
                ALU.add,
            )
            # multiplicative key mask (scales V rows and the ones column)
            nc.vector.tensor_scalar(vloc[:, j, :, :], vloc[:, j, :, :],
                                    m01c[:, j : j + 1], None, ALU.mult)
        agv_in = t["ag_in"][KBLK : KBLK + VBLK].rearrange("(j p w) -> p j w", p=P,
                                                          w=H * (DH + 1))
        nc.sync.dma_start(agv_in, vloc[:].rearrange("p j h w -> p j (h w)"))

        # exchange K/V across the 4 cores of this batch
        nc.gpsimd.collective_compute(
            "AllGather",
            ALU.bypass,
            ins=[t["ag_in"][:]],
            outs=[t["ag_out"][:]],
            replica_groups=[[0, 1, 2, 3], [4, 5, 6, 7]],
        )

        # Q projection overlaps the collective
        wq_t = pw.tile([P, NDC, D], BF16, tag="wm")
        nc.sync.dma_start(wq_t[:], t["wq"].rearrange("(c p) n -> p c n", p=P))
        qT = pqkv.tile([P, NDC, TQ], BF16, name="qT")
        for n in range(NDC):
            ps = ps_mm.tile([P, 512], FP32, tag="mm")
            for c in range(NDC):
                nc.tensor.matmul(ps[:], wq_t[:, c, ts(n, P)], hT[:, c, :],
                                 start=(c == 0), stop=(c == NDC - 1))
            nc.vector.tensor_scalar(qT[:, n, :], ps[:], 0.125, bq_s[:, n : n + 1],
                                    ALU.mult, ALU.add)
        pools.close("wproj")
        pools.close("h")

        # wo prefetch overlaps attention (released wproj zone gates the DMA)
        pwo = pools.open("wo", bufs=1)
        wo_t = pwo.tile([P, NDC, D], BF16, name="wo_t")
        nc.sync.dma_start(wo_t[:], t["wo"].rearrange("(c p) n -> p c n", p=P))

        # gathered K/V back to SBUF: rank r holds global tokens [r*TQ,(r+1)*TQ)
        BLK = KBLK + VBLK
        kT = pqkv.tile([P, NDC, 4, TQ], BF16, name="kT")
        for r in range(4):
            nc.sync.dma_start(
                kT[:, :, r, :],
                t["ag_out"][r * BLK : r * BLK + KBLK].rearrange("(c p q) -> p c q",
                                                                p=P, q=TQ),
            )
        vaug = pqkv.tile([P, 4, TQ // P, H, DH + 1], BF16, name="vaug")
        for r in range(4):
            nc.sync.dma_start(
                vaug[:, r, :, :, :].rearrange("p j h w -> p j (h w)"),
                t["ag_out"][r * BLK + KBLK : (r + 1) * BLK].rearrange(
                    "(j p w) -> p j w", p=P, w=H * (DH + 1)),
            )

        # ---------------- phase C: attention (per head, sw-pipelined) ----------------
        ppt = pools.open("pt", bufs=2, side="right")
        pattn = pools.open("attn", bufs=1)
        attnT = pattn.tile([P, NDC, TQ], BF16, name="attnT")

        pt_tiles = [None] * H

        def emit_scores(h):
            pof = DH * (h % 2)
            ch = h // 2
            pt = ppt.tile([P, NKC, TQ], BF16, tag="pt")
            pt_tiles[h] = pt
            for kp in range(NKC // 2):
                kc0, kc1 = 2 * kp, 2 * kp + 1
                ps = ps_mm.tile([P, 1024], FP32, tag="mm")
                nc.tensor.matmul(ps[:, 0:512],
                                 kT[pof : pof + DH, ch, kc0 // 4, ts(kc0 % 4, P)],
                                 qT[pof : pof + DH, ch, :], start=True, stop=True)
                nc.tensor.matmul(ps[:, 512:1024],
                                 kT[pof : pof + DH, ch, kc1 // 4, ts(kc1 % 4, P)],
                                 qT[pof : pof + DH, ch, :], start=True, stop=True)
                nc.scalar.activation(pt[:, 2 * kp : 2 * kp + 2, :],
                                     ps[:].rearrange("p (a b) -> p a b", b=512), AF.Exp)

        def emit_attnv(h):
            pof = DH * (h % 2)
            ch = h // 2
            pt = pt_tiles[h]
            pa = ps_pa.tile([DH + 1, TQ], FP32, tag="pa")
            for kc in range(NKC):
                nc.tensor.matmul(pa[:], vaug[:, kc // 4, kc % 4, h, :], pt[:, kc, :],
                                 start=(kc == 0), stop=(kc == NKC - 1))
            drow = pattn.tile([1, TQ], FP32, tag="drow")
            nc.vector.tensor_copy(drow[:], pa[DH : DH + 1, :])
            pb = ps_bc.tile([P, 512], FP32, tag="bc")
            nc.tensor.matmul(pb[0:DH, :], ones_row[:, 0:DH], drow[:], start=True, stop=True)
            rdb = pattn.tile([DH, TQ], FP32, tag="rdb")
            nc.vector.reciprocal(rdb[:], pb[0:DH, :])
            nc.vector.tensor_mul(attnT[pof : pof + DH, ch, :], pa[0:DH, :], rdb[:])

        # software pipeline: scores(h+1) overlaps exp(h) / attnV(h)
        emit_scores(0)
        for h in range(H):
            if h + 1 < H:
                emit_scores(h + 1)
            emit_attnv(h)
            pt_tiles[h] = None
        pools.close("pt")
        pools.close("qkv")

        # ---------------- phase D: output projection + residual ----------------
        pres = pools.open("res1", bufs=1, side="right")
        res1T = pres.tile([P, NDC, TQ], FP32, name="res1T")
        for n in range(NDC):
            ps = ps_mm.tile([P, 512], FP32, tag="mm")
            for c in range(NDC):
                nc.tensor.matmul(ps[:], wo_t[:, c, ts(n, P)], attnT[:, c, :],
                                 start=(c == 0), stop=(c == NDC - 1))
            nc.vector.tensor_scalar(res1T[:, n, :], ps[:], bo_c[:, n : n + 1], None, ALU.add)
            nc.vector.tensor_add(res1T[:, n, :], res1T[:, n, :], hq[:, n, :])
        pools.close("attn")
        pools.close("wo")
        pools.close("hq")

        # ---------------- phase E: LN2 (TQ tokens) ----------------
        plnE = pools.open("lnE", bufs=2, side="right")
        h2T = plnE.tile([P, NDC, TQ], BF16, name="h2T")
        ln_tile(res1T, slice(0, TQ), h2T, None, g2, b2l, plnE, "E")

        # ---------------- phase F: FFN (interleaved halves) ----------------
        pg1 = pools.open("g1", bufs=1)
        pwf1 = pools.open("wf1", bufs=1)
        pwf2 = pools.open("wf2", bufs=1)
        pout = pools.open("out", bufs=1)

        g1T = pg1.tile([P, NFC, TQ], BF16, name="g1T")
        outT = pout.tile([P, NDC, TQ], FP32, name="outT")
        w1_r = t["w1"].rearrange("(c p) f -> p c f", p=P)
        w2_r = t["w2"].rearrange("(f p) d -> p f d", p=P)

        for hf in range(2):
            w1h = pwf1.tile([P, NDC, DFF // 2], BF16, tag="w1h")
            nc.sync.dma_start(w1h[:], w1_r[:, :, ts(hf, DFF // 2)])
            for fl in range(NFC // 2):
                fc = hf * (NFC // 2) + fl
                ps = ps_mm.tile([P, 512], FP32, tag="mm")
                for c in range(NDC):
                    nc.tensor.matmul(ps[:], w1h[:, c, ts(fl, P)], h2T[:, c, :],
                                     start=(c == 0), stop=(c == NDC - 1))
                nc.scalar.activation(g1T[:, fc, :], ps[:], AF.Relu,
                                     bias=b1_c[:, fc : fc + 1], scale=1.0)
            # second FFN matmul for this half of d_ff
            w2h = pwf2.tile([P, NFC // 2, D], BF16, tag="w2h")
            nc.sync.dma_start(w2h[:], w2_r[:, ts(hf, NFC // 2), :])
            for n in range(NDC):
                ps = ps_mm.tile([P, 512], FP32, tag="mm")
                for fl in range(NFC // 2):
                    fc = hf * (NFC // 2) + fl
                    nc.tensor.matmul(ps[:], w2h[:, fl, ts(n, P)], g1T[:, fc, :],
                                     start=(fl == 0), stop=(fl == NFC // 2 - 1))
                if hf == 0:
                    nc.vector.tensor_scalar(outT[:, n, :], ps[:], b2_c[:, n : n + 1],
                                            None, ALU.add)
                else:
                    nc.vector.tensor_add(outT[:, n, :], outT[:, n, :], ps[:])
        pools.close("lnE")

        # final residual: out = res1 + ffn
        for n in range(NDC):
            nc.vector.tensor_add(outT[:, n, :], outT[:, n, :], res1T[:, n, :])
        pools.close("res1")

        # ---------------- transpose to token-major and store ----------------
        out_sb = pout.tile([P, TQ // P, D], FP32, name="out_sb")
        for n in range(NDC):
            for qc in range(TQ // P):
                pt_ = ps_bc.tile([P, P], FP32, tag="bc")
                nc.tensor.transpose(pt_[:], outT[:, n, ts(qc, P)], ident[:])
                nc.scalar.copy(out_sb[:, qc, ts(n, P)], pt_[:])
        nc.sync.dma_start(t["out"].rearrange("(qc p) d -> p qc d", p=P), out_sb[:])

        pools.close_all()


def _build_nc(split=True):
    nc = bass.Bass("TRN2", target_bir_lowering=False, debug=False, num_devices=NCORES)

    t = {}

    def inp(name, shape, dtype=FP32):
        t[name] = nc.dram_tensor(name, shape, dtype, kind="ExternalInput").ap()

    inp("xT", [D, TQ])
    inp("m01", [TQ])
    for nm in ("ln1_g", "ln1_b", "ln2_g", "ln2_b", "bq", "bk", "bv", "bo", "b2"):
        inp(nm, [D])
    inp("b1", [DFF])
    for nm in ("wq", "wk", "wv", "wo"):
        inp(nm, [D, D], BF16)
    inp("w1", [D, DFF], BF16)
    inp("w2", [DFF, D], BF16)
    t["out"] = nc.dram_tensor("out", [TQ, D], FP32, kind="ExternalOutput").ap()
    KBLK = D * TQ                 # bf16 elements of the K part per rank
    VBLK = TQ * H * (DH + 1)      # bf16 elements of the V part per rank
    t["ag_in"] = nc.dram_tensor("ag_in", [KBLK + VBLK], BF16).ap()
    t["ag_out"] = nc.dram_tensor("ag_out", [4 * (KBLK + VBLK)], BF16).ap()

    _emit(nc, t)
    if split:
        _split_multi_waits(nc)
    return nc


_NC_CACHE = None


def _get_nc():
    global _NC_CACHE
    if _NC_CACHE is None:
        _NC_CACHE = _build_nc()
    return _NC_CACHE


def make_in_maps(x, mask, ln1_g, ln1_b, ln2_g, ln2_b,
                 wq, bq, wk, bk, wv, bv, wo, bo, w1, b1, w2, b2):
    """Build the 8 per-core input dicts from the full-size inputs."""
    bf = ml_dtypes.bfloat16
    shared = {
        "ln1_g": np.ascontiguousarray(ln1_g, np.float32),
        "ln1_b": np.ascontiguousarray(ln1_b, np.float32),
        "ln2_g": np.ascontiguousarray(ln2_g, np.float32),
        "ln2_b": np.ascontiguousarray(ln2_b, np.float32),
        "bq": np.ascontiguousarray(bq, np.float32),
        "bk": np.ascontiguousarray(bk, np.float32),
        "bv": np.ascontiguousarray(bv, np.float32),
        "bo": np.ascontiguousarray(bo, np.float32),
        "b1": np.ascontiguousarray(b1, np.float32),
        "b2": np.ascontiguousarray(b2, np.float32),
        "wq": np.ascontiguousarray(np.asarray(wq, np.float32).astype(bf)),
        "wk": np.ascontiguousarray(np.asarray(wk, np.float32).astype(bf)),
        "wv": np.ascontiguousarray(np.asarray(wv, np.float32).astype(bf)),
        "wo": np.ascontiguousarray(np.asarray(wo, np.float32).astype(bf)),
        "w1": np.ascontiguousarray(np.asarray(w1, np.float32).astype(bf)),
        "w2": np.ascontiguousarray(np.asarray(w2, np.float32).astype(bf)),
    }
    x = np.asarray(x, np.float32)
    mask = np.asarray(mask)
    in_maps = []
    for core in range(NCORES):
        b, qb = divmod(core, NCORES // 2)
        xb = x[b, qb * TQ : (qb + 1) * TQ]                     # (TQ, D)
        m01 = (np.asarray(mask[b, qb * TQ : (qb + 1) * TQ]) != 0).astype(np.float32)
        in_maps.append({
            **shared,
            "xT": np.ascontiguousarray(xb.T),
            "m01": np.ascontiguousarray(m01),
        })
    return in_maps


def kernel(**inputs):
    nc = _get_nc()
    in_maps = make_in_maps(**inputs)
    res = run_bass_kernel_spmd(nc, in_maps, core_ids=list(range(NCORES)))
    Bsz, S, _ = np.asarray(inputs["x"]).shape
    out = np.empty((Bsz, S, D), np.float32)
    for core in range(NCORES):
        b, qb = divmod(core, NCORES // 2)
        out[b, qb * TQ : (qb + 1) * TQ, :] = res.results[core]["out"]
    return out
